# revision 1
# baseline (speedup 1.0000x reference)
"""Trainium2 Bass kernel for nn_EntityResolutionProcessor.

Strategy: data-parallel over mentions (M=1024 -> 128/core on 8 cores).
On-device per core:
  phase0: blocked cumsum of text -> csum scratch in DRAM (f32); indirect-DMA
          gather of 4 csum rows per mention; mention/context means (f32);
          weights + candidates converted to bf16 DRAM scratch.
  per-mention: feature-major projections (relik-W1a, q/k/v, uni-U1a), s_aa.
  8 macro-tiles of 512 pairs: candidate DMA-transpose, q/k/v projections,
          relik/unirel heads, 2-token attention via sigmoid softmax,
          wo + LN1, FFN, LN2+cosine fused via sufficient statistics.
Activations feature-major [feat->6x128 partitions, pairs]. Matmul operands
bf16 (fp32 psum accumulate); cumsum/means/layernorm lane math in fp32.
"""

from contextlib import ExitStack

import ml_dtypes
import numpy as np

import concourse.bass as bass
import concourse.mybir as mybir
import concourse.tile as tile
from concourse import bacc, bass_isa, bass_utils
from concourse.bass import IndirectOffsetOnAxis, ds, ts

S, D, M, K, H = 4096, 768, 1024, 32, 8
DH = D // H
CTX = 10
NCORES = 8
P = 128
FC = D // P                     # 6 feature chunks
HFC = 4 * D // P                # 24 ffn hidden chunks
M_LOC = M // NCORES             # 128 mentions per core
PAIRS = M_LOC * K               # 4096 pairs per core
NP = 512                        # pairs per macro tile
G = NP // K                     # 16 mentions per macro tile
NMACRO = PAIRS // NP            # 8
NCH = S // P                    # 32 text chunks
ISQ = 1.0 / float(np.sqrt(np.float32(DH)))
EPS_LN = 1e-5
EPS_COS = 1e-8

F32 = mybir.dt.float32
BF16 = mybir.dt.bfloat16
I32 = mybir.dt.int32
AF = mybir.ActivationFunctionType
ALU = mybir.AluOpType

_NC_CACHE = {}


def _gk(ap):
    """view a [128, NP] AP as [128, G, K]"""
    return ap.rearrange("p (g k) -> p g k", g=G)


def _feat_major(w_ap):
    """[in, out] dram AP -> [128, in//128, out] (partition = in % 128)"""
    return w_ap.rearrange("(i p) o -> p i o", p=P)


def _vec6(v_ap, n=FC):
    """[D] dram AP -> [128, n] per-feature layout"""
    return v_ap.rearrange("(i p) -> p i", p=P)


def _build_nc():
    nc = bacc.Bacc(
        "TRN2", target_bir_lowering=False, debug=False, num_devices=NCORES
    )

    def inp(name, shape, dtype=F32):
        return nc.dram_tensor(name, list(shape), dtype, kind="ExternalInput").ap()

    t = {}
    t["txt"] = inp("txt", [S, D])
    t["cand"] = inp("cand", [PAIRS, D])
    t["idx"] = inp("idx", [P, 4], I32)
    t["invl"] = inp("invl", [P, 2])
    t["seltab"] = inp("seltab", [NCH, 2, P])
    t["tri"] = inp("tri", [P, P])          # upper-tri incl (lhsT of L)
    t["tri32"] = inp("tri32", [NCH, NCH])  # strict upper (lhsT of strict L)
    t["ident"] = inp("ident", [P, P])
    t["identb"] = inp("identb", [P, P], BF16)
    t["zrow"] = inp("zrow", [1, D])
    t["hmat"] = inp("hmat", [D, H], BF16)  # head indicator
    t["i8neg"] = inp("i8neg", [H, H], BF16)

    for n, shp in [("relik_w1", [2 * D, D]), ("relik_b1", [D]),
                   ("relik_w2", [D, 1]), ("relik_b2", [1, 1]),
                   ("wq", [D, D]), ("bq", [D]), ("wk", [D, D]), ("bk", [D]),
                   ("wv", [D, D]), ("bv", [D]), ("wo", [D, D]), ("bo", [D]),
                   ("ln1_g", [D]), ("ln1_b", [D]),
                   ("ffn_w1", [D, 4 * D]), ("ffn_b1", [4 * D]),
                   ("ffn_w2", [4 * D, D]), ("ffn_b2", [D]),
                   ("ln2_g", [D]), ("ln2_b", [D]),
                   ("uni_w1", [2 * D, D]), ("uni_b1", [D]),
                   ("uni_w2", [D, D]), ("uni_b2", [1, D])]:
        t[n] = inp(n, shp)

    t["out"] = nc.dram_tensor("out", [3, PAIRS], F32, kind="ExternalOutput").ap()
    t["csum"] = nc.dram_tensor("csum_scratch", [S + 1, D], F32).ap()
    # bf16 scratch: candidates + streamed weights (strip-major layouts)
    t["cand_bf"] = nc.dram_tensor("cand_bf", [PAIRS, D], BF16).ap()
    for n, noc, nic in [("wq_bf", FC, FC), ("wk_bf", FC, FC),
                        ("wv_bf", FC, FC), ("wo_bf", FC, FC),
                        ("w1b_bf", FC, FC), ("u1b_bf", FC, FC),
                        ("fw1_bf", HFC, FC), ("fw2_bf", FC, HFC)]:
        t[n] = nc.dram_tensor(n, [noc, P, nic * P], BF16).ap()

    with tile.TileContext(nc) as tc:
        _body(nc, tc, t)
    nc.compile()
    return nc


def _body(nc, tc, t):
    with ExitStack() as _ctx:
        _body_inner(nc, tc, t, _ctx)


def _body_inner(nc, tc, t, _ctx):
    mm = lambda *a, **k: nc.tensor.matmul(*a, **k)

    # ---------------- pools ----------------
    psum = _ctx.enter_context(tc.tile_pool(name="psum", bufs=1, space="PSUM"))
    res = _ctx.enter_context(tc.tile_pool(name="res", bufs=1))

    def ps_mm(shape=(P, NP), dtype=F32):
        return psum.tile(list(shape), dtype, tag="mm", bufs=3,
                         padded_shape=[P, NP], name="ps_mm")

    def ps_score():
        return psum.tile([8, NP], F32, tag="score", bufs=1, name="ps_score")

    def ps_stat():
        # stats tile: MM groups land at base partitions 0 and 32
        return psum.tile([P, NP], F32, tag="stat", bufs=2, name="ps_stat")

    def ps_head():
        return psum.tile([1, NP], F32, tag="head", bufs=2, name="ps_head")

    # ---------------- resident constants ----------------
    def load_res(name, ap_src, shape, dtype=F32, conv=False):
        tl = res.tile(list(shape), dtype, name=name)
        nc.gpsimd.dma_start(tl[:], ap_src)
        return tl

    tri_sb = load_res("tri_sb", t["tri"][:], [P, P])
    tri32_sb = load_res("tri32_sb", t["tri32"][:], [NCH, NCH])
    ident_sb = load_res("ident_sb", t["ident"][:], [P, P])
    identb_sb = load_res("identb_sb", t["identb"][:], [P, P], BF16)
    i8neg_sb = load_res("i8neg_sb", t["i8neg"][:], [H, H], BF16)
    # H in two layouts: lhsT for head-reduce [128,6,8]; lhsT for bcast [8,6,128]
    h_sb = load_res("h_sb", t["hmat"].rearrange("(c p) h -> p c h", p=P),
                    [P, FC, H], BF16)
    ht_sb = load_res("ht_sb", t["hmat"].rearrange("(c p) h -> h c p", p=P),
                     [H, FC, P], BF16)
    negh_sb = res.tile([P, FC, H], BF16, name="negh_sb")
    nc.vector.tensor_scalar_mul(negh_sb[:], h_sb[:], -1.0)

    idx_sb = load_res("idx_sb", t["idx"][:], [P, 4], I32)
    invl_sb = load_res("invl_sb", t["invl"][:], [P, 2])
    sel_sb = load_res("sel_sb", t["seltab"][:], [NCH, 2, P])

    bq_sb = load_res("bq_sb", _vec6(t["bq"]), [P, FC])
    bk_sb = load_res("bk_sb", _vec6(t["bk"]), [P, FC])
    bv_sb = load_res("bv_sb", _vec6(t["bv"]), [P, FC])
    bo_sb = load_res("bo_sb", _vec6(t["bo"]), [P, FC])
    rb1_sb = load_res("rb1_sb", _vec6(t["relik_b1"]), [P, FC])
    ub1_sb = load_res("ub1_sb", _vec6(t["uni_b1"]), [P, FC])
    fb1_sb = load_res("fb1_sb", _vec6(t["ffn_b1"], HFC), [P, HFC])
    fb2_sb = load_res("fb2_sb", _vec6(t["ffn_b2"]), [P, FC])
    l1g_sb = load_res("l1g_sb", _vec6(t["ln1_g"]), [P, FC])
    l1b_sb = load_res("l1b_sb", _vec6(t["ln1_b"]), [P, FC])
    l2g_sb = load_res("l2g_sb", _vec6(t["ln2_g"]), [P, FC])
    l2b_sb = load_res("l2b_sb", _vec6(t["ln2_b"]), [P, FC])
    rw2_sb = load_res("rw2_sb",
                      t["relik_w2"].rearrange("(c p) o -> p c o", p=P),
                      [P, FC, 1], BF16, conv=True)
    rb2_sb = load_res("rb2_sb", t["relik_b2"][:], [1, 1])

    ones_sb = res.tile([P, 1], BF16, name="ones_sb")
    nc.vector.memset(ones_sb[:], 1.0)
    onesf_sb = res.tile([P, 1], F32, name="onesf_sb")
    nc.vector.memset(onesf_sb[:], 1.0)
    ones_row = res.tile([1, P], BF16, name="ones_row")
    nc.vector.memset(ones_row[:], 1.0)

    # stats lhsT [128, 6, 3]: cols = [1, g2^2, g2*b2] per feature chunk
    sl3_sb = res.tile([P, FC, 3], BF16, name="sl3_sb")
    g2sq_sb = res.tile([P, FC], F32, name="g2sq_sb")
    g2b2_sb = res.tile([P, FC], F32, name="g2b2_sb")
    nc.vector.tensor_mul(g2sq_sb[:], l2g_sb[:], l2g_sb[:])
    nc.vector.tensor_mul(g2b2_sb[:], l2g_sb[:], l2b_sb[:])
    for c in range(FC):
        nc.vector.tensor_copy(sl3_sb[:, c, 0:1], ones_sb[:])
        nc.vector.tensor_copy(sl3_sb[:, c, 1:2], g2sq_sb[:, c:c + 1])
        nc.vector.tensor_copy(sl3_sb[:, c, 2:3], g2b2_sb[:, c:c + 1])

    # scalar reductions of bias/gain vectors (each -> [1,1] on partition 0)
    def vec_sum(name, vecs):
        tmp = res.tile([P, FC], F32, name=name + "_t")
        if len(vecs) == 1:
            nc.vector.tensor_copy(tmp[:], vecs[0][:])
        else:
            nc.vector.tensor_mul(tmp[:], vecs[0][:], vecs[1][:])
            for v in vecs[2:]:
                nc.vector.tensor_mul(tmp[:], tmp[:], v[:])
        red = res.tile([P, 1], F32, name=name + "_r")
        nc.vector.tensor_reduce(red[:], tmp[:], axis=mybir.AxisListType.X,
                                op=ALU.add)
        pR = ps_head()
        mm(pR[:, 0:1], red[:], onesf_sb[:], start=True, stop=True)
        arr = res.tile([1, 1], F32, name=name)
        nc.vector.tensor_copy(arr[:], pR[:, 0:1])
        return arr[0:1, 0:1]

    s_bo = vec_sum("s_bo", [bo_sb])
    s_fb2 = vec_sum("s_fb2", [fb2_sb])
    s_g2 = vec_sum("s_g2", [l2g_sb, l2g_sb])
    s_gb = vec_sum("s_gb", [l2g_sb, l2b_sb])
    s_bb = vec_sum("s_bb", [l2b_sb, l2b_sb])
    s_g2f = vec_sum("s_g2f", [l2g_sb, l2g_sb, fb2_sb])
    s_gbf = vec_sum("s_gbf", [l2g_sb, l2b_sb, fb2_sb])

    u2rs_sb = res.tile([P, FC], BF16, name="u2rs_sb")
    b2m_sb = res.tile([1, 1], F32, name="b2m_sb")

    # per-mention outputs (feature-major [128, 6, 128])
    m_T = res.tile([P, FC, P], F32, name="m_T")     # f32: residual source
    m_Tb = res.tile([P, FC, P], BF16, name="m_Tb")  # bf16: matmul rhs
    c_Tb = res.tile([P, FC, P], BF16, name="c_Tb")
    m_q = res.tile([P, FC, P], BF16, name="m_q")
    m_k = res.tile([P, FC, P], BF16, name="m_k")
    m_v = res.tile([P, FC, P], BF16, name="m_v")
    m_relik = res.tile([P, FC, P], BF16, name="m_relik")
    c_uni = res.tile([P, FC, P], BF16, name="c_uni")
    s_aa_sb = res.tile([H, P], BF16, name="s_aa_sb")

    # ================= phase 0: csum + gather + bf16 conversion ==========
    with tc.tile_pool(name="p0", bufs=1) as p0:
        # uni_w2 row-sums (once)
        u2_sb = p0.tile([P, FC, D], F32, name="u2_sb")
        nc.gpsimd.dma_start(u2_sb[:], _feat_major(t["uni_w2"]))
        u2r_f = p0.tile([P, FC], F32, name="u2r_f")
        nc.vector.tensor_reduce(u2r_f[:], u2_sb[:],
                                axis=mybir.AxisListType.X, op=ALU.add)
        nc.vector.tensor_copy(u2rs_sb[:], u2r_f[:])
        ub2_sb = p0.tile([1, D], F32, name="ub2_sb")
        nc.gpsimd.dma_start(ub2_sb[:], t["uni_b2"][:])
        b2r = p0.tile([1, 1], F32, name="b2r")
        nc.vector.tensor_reduce(b2r[:], ub2_sb[:], axis=mybir.AxisListType.X,
                                op=ALU.add)
        nc.scalar.activation(b2m_sb[:], b2r[:], AF.Copy, scale=1.0 / D)

        # ---- bf16 weight conversion into strip-major scratch ----
        for src_ap, dst, noc, nic in [
            (_feat_major(t["wq"]), t["wq_bf"], FC, FC),
            (_feat_major(t["wk"]), t["wk_bf"], FC, FC),
            (_feat_major(t["wv"]), t["wv_bf"], FC, FC),
            (_feat_major(t["wo"]), t["wo_bf"], FC, FC),
            (_feat_major(t["relik_w1"][D:]), t["w1b_bf"], FC, FC),
            (_feat_major(t["uni_w1"][D:]), t["u1b_bf"], FC, FC),
            (_feat_major(t["ffn_w1"]), t["fw1_bf"], HFC, FC),
            (_feat_major(t["ffn_w2"]), t["fw2_bf"], FC, HFC),
        ]:
            for oc in range(noc):
                nc.gpsimd.dma_start(
                    dst[oc].rearrange("p (i q) -> p i q", q=P),
                    src_ap[:, :, ts(oc, P)])

        # ---- candidates to bf16 (converting DRAM->DRAM DMA) ----
        for c in range(4):
            q = PAIRS // 4
            nc.gpsimd.dma_start(t["cand_bf"][c * q:(c + 1) * q, :],
                                t["cand"][c * q:(c + 1) * q, :])

        # ---- cumsum ----
        totals_sb = p0.tile([NCH, D], F32, name="totals_sb")
        nc.gpsimd.dma_start(t["csum"][0:1, :], t["zrow"][:])

        for c in range(NCH):
            txt_c = p0.tile([P, D], F32, tag="txtc", bufs=3, name="txt_c")
            nc.gpsimd.dma_start(txt_c[:], t["txt"][c * P:(c + 1) * P, :])
            pre_sb = p0.tile([P, D], F32, tag="pre", bufs=3, name="pre_sb")
            for half in range(2):
                sl = ds(half * 384, 384)
                pA = ps_mm((P, 384))
                mm(pA[:], tri_sb[:], txt_c[:, sl], start=True, stop=True)
                nc.any.tensor_copy(pre_sb[:, sl], pA[:])
            nc.gpsimd.dma_start(t["csum"][1 + c * P: 1 + (c + 1) * P, :],
                                pre_sb[:])
            nc.gpsimd.dma_start(totals_sb[c:c + 1, :], pre_sb[P - 1:P, :])

        offs_sb = p0.tile([NCH, D], F32, name="offs_sb")
        for half in range(2):
            sl = ds(half * 384, 384)
            pA = ps_mm((NCH, 384))
            mm(pA[:], tri32_sb[:], totals_sb[:, sl], start=True, stop=True)
            nc.any.tensor_copy(offs_sb[:, sl], pA[:])

        # ---- gathers + means ----
        gath = []
        for j in range(4):
            g_t = p0.tile([P, D], F32, tag=f"g{j}", name=f"g_{j}")
            nc.gpsimd.indirect_dma_start(
                out=g_t[:], out_offset=None, in_=t["csum"][:],
                in_offset=IndirectOffsetOnAxis(ap=idx_sb[:, j:j + 1], axis=0),
            )
            gath.append(g_t)

        def mean_tile(out_name, gp, gm, selcol, inv_col):
            o_t = p0.tile([P, D], F32, name=out_name)
            dif = p0.tile([P, D], F32, tag="dif", bufs=2, name="dif")
            nc.vector.tensor_tensor(dif[:], gath[gp][:], gath[gm][:],
                                    op=ALU.subtract)
            for half in range(2):
                sl = ds(half * 384, 384)
                pA = ps_mm((P, 384))
                mm(pA[:], sel_sb[:, selcol, :], offs_sb[:, sl],
                   start=True, stop=True)
                nc.vector.tensor_tensor(o_t[:, sl], pA[:], dif[:, sl],
                                        op=ALU.add)
            nc.vector.tensor_scalar_mul(o_t[:], o_t[:],
                                        invl_sb[:, inv_col:inv_col + 1])
            return o_t

        mention_rm = mean_tile("mention_rm", 0, 1, 0, 0)
        ctx_rm = mean_tile("ctx_rm", 2, 3, 1, 1)

        for src, dstf, dstb in ((mention_rm, m_T, m_Tb),
                                (ctx_rm, None, c_Tb)):
            for fc in range(FC):
                pT = ps_mm((P, P))
                nc.tensor.transpose(pT[:], src[:, ts(fc, P)], ident_sb[:])
                if dstf is not None:
                    nc.vector.tensor_copy(dstf[:, fc, :], pT[:])
                nc.any.tensor_copy(dstb[:, fc, :], pT[:])

    # ================= pools for the main phase =================
    wts = _ctx.enter_context(tc.tile_pool(name="wts", bufs=1))
    act = _ctx.enter_context(tc.tile_pool(name="act", bufs=1))
    lane = _ctx.enter_context(tc.tile_pool(name="lane", bufs=1))

    def load_strip(bf_dram, oc):
        """stream bf16 weight strip [128, 6, 128] for out-chunk oc"""
        st = wts.tile([P, FC, P], BF16, tag="wstrip", bufs=6, name="w_strip")
        nc.gpsimd.dma_start(st[:],
                          bf_dram[oc].rearrange("p (i q) -> p i q", q=P))
        return st

    def load_strip_conv(w_fm_ap, oc):
        """one-shot converting load (per-mention phase)"""
        st = wts.tile([P, FC, P], BF16, tag="wstrip", bufs=6, name="w_strip")
        nc.gpsimd.dma_start(st[:], w_fm_ap[:, :, ts(oc, P)])
        return st

    def unit(tag, name, bufs=1):
        return act.tile([P, FC, NP], BF16, tag=tag, bufs=bufs, name=name)

    def chunk_t(name):
        return act.tile([P, NP], BF16, tag="tt", bufs=3, name=name)

    # ---------- per-mention projections (bf16, N=128) ----------
    for w_ap, b_sb, out_t, src in (
        (_feat_major(t["wq"]), bq_sb, m_q, m_Tb),
        (_feat_major(t["wk"]), bk_sb, m_k, m_Tb),
        (_feat_major(t["wv"]), bv_sb, m_v, m_Tb),
        (_feat_major(t["relik_w1"][:D]), rb1_sb, m_relik, m_Tb),
        (_feat_major(t["uni_w1"][:D]), ub1_sb, c_uni, c_Tb),
    ):
        for oc in range(FC):
            st = load_strip_conv(w_ap, oc)
            pA = ps_mm((P, P))
            for ic in range(FC):
                mm(pA[:], st[:, ic, :], src[:, ic, :],
                   start=(ic == 0), stop=(ic == FC - 1))
            nc.scalar.activation(out_t[:, oc, :], pA[:], AF.Identity,
                                 bias=b_sb[:, oc:oc + 1])

    # s_aa [8, 128]
    mprod = wts.tile([P, FC, P], BF16, tag="wstrip", bufs=6, name="mprod")
    for c in range(FC):
        nc.vector.tensor_mul(mprod[:, c, :], m_q[:, c, :], m_k[:, c, :])
    pS = ps_score()
    for c in range(FC):
        mm(pS[:, :P], h_sb[:, c, :], mprod[:, c, :],
           start=(c == 0), stop=(c == FC - 1))
    nc.any.tensor_copy(s_aa_sb[:], pS[:, :P])

    # ================= macro-tile loop =================
    for mt in range(NMACRO):
        g0 = mt * G
        gsl = ds(g0, G)

        lane_seq = [0]

        def lane_t(name, parts=1):
            lane_seq[0] += 1
            return lane.tile([parts, NP], F32, tag=name, bufs=1,
                             name=f"{name}_{lane_seq[0]}")

        def mview(mt_tile, c):
            """mention-side bcast view [128, G, K]"""
            return mt_tile[:, c, gsl, None].to_broadcast([P, G, K])

        # ---- candidate load + PE transpose (bf16) ----
        cand_rm = act.tile([P, 4, D], BF16, tag="cand_rm", bufs=1,
                           name="cand_rm")
        nc.gpsimd.dma_start(
            cand_rm[:],
            t["cand_bf"].rearrange("(q p) d -> p q d", p=P)[:, ds(4 * mt, 4), :])
        candT = unit("candT", "candT")
        for fc in range(FC):
            pT = ps_mm(dtype=BF16)
            for pc in range(4):
                nc.tensor.transpose(pT[:, ts(pc, P)],
                                    cand_rm[:, pc, ts(fc, P)], identb_sb[:])
            nc.vector.tensor_copy(candT[:, fc, :], pT[:])

        # ---- k/v projections ----
        k_b = unit("B", "k_b")
        v_b = unit("C", "v_b")
        for wbf, b_sb, out_t in ((t["wk_bf"], bk_sb, k_b),
                                 (t["wv_bf"], bv_sb, v_b)):
            for oc in range(FC):
                st = load_strip(wbf, oc)
                pA = ps_mm()
                for ic in range(FC):
                    mm(pA[:], st[:, ic, :], candT[:, ic, :],
                       start=(ic == 0), stop=(ic == FC - 1))
                nc.scalar.activation(out_t[:, oc, :], pA[:], AF.Identity,
                                     bias=b_sb[:, oc:oc + 1])

        # ---- relik / unirel heads ----
        for wbf, madd, hname, wv2, bias_ap, outrow, fn, scale in (
            (t["w1b_bf"], m_relik, "h_r", rw2_sb, rb2_sb[:], 0,
             AF.Identity, 1.0),
            (t["u1b_bf"], c_uni, "h_u", u2rs_sb, b2m_sb[:], 2,
             AF.Sigmoid, 1.0 / D),
        ):
            h_head = unit("hh", hname, bufs=2)
            for oc in range(FC):
                st = load_strip(wbf, oc)
                pA = ps_mm()
                for ic in range(FC):
                    mm(pA[:], st[:, ic, :], candT[:, ic, :],
                       start=(ic == 0), stop=(ic == FC - 1))
                nc.vector.tensor_tensor(_gk(h_head[:, oc, :]), _gk(pA[:]),
                                        mview(madd, oc), op=ALU.add)
                nc.scalar.activation(h_head[:, oc, :], h_head[:, oc, :],
                                     AF.Relu)
            pH = ps_head()
            for c in range(FC):
                if wv2 is rw2_sb:
                    lhsT = wv2[:, c, :]
                else:
                    lhsT = wv2[:, c:c + 1]
                mm(pH[:], lhsT, h_head[:, c, :],
                   start=(c == 0), stop=(c == FC - 1))
            osl = lane_t("osl_" + hname)
            nc.scalar.activation(osl[:], pH[:], fn, bias=bias_ap, scale=scale)
            nc.gpsimd.dma_start(t["out"][outrow:outrow + 1, ts(mt, NP)], osl[:])

        # ---- attention scores ----
        pAB = ps_score()
        for c in range(FC):
            pr1 = chunk_t("pr1")
            nc.vector.tensor_tensor(_gk(pr1[:]), _gk(k_b[:, c, :]),
                                    mview(m_q, c), op=ALU.mult)
            mm(pAB[:], h_sb[:, c, :], pr1[:], start=(c == 0), stop=False)
        mm(pAB[:], i8neg_sb[:],
           s_aa_sb[:, gsl, None].to_broadcast([H, G, K]),
           start=False, stop=True)
        p_ab = act.tile([H, NP], BF16, tag="p_ab", bufs=2, name="p_ab")
        nc.scalar.activation(p_ab[:], pAB[:], AF.Sigmoid, scale=ISQ)

        pBA = ps_score()
        first = True
        for c in range(FC):
            stq = load_strip(t["wq_bf"], c)
            pQ = ps_mm()
            for ic in range(FC):
                mm(pQ[:], stq[:, ic, :], candT[:, ic, :],
                   start=(ic == 0), stop=(ic == FC - 1))
            q_c = chunk_t("q_c")
            nc.scalar.activation(q_c[:], pQ[:], AF.Identity,
                                 bias=bq_sb[:, c:c + 1])
            pr2 = chunk_t("pr2")
            nc.vector.tensor_tensor(_gk(pr2[:]), _gk(q_c[:]), mview(m_k, c),
                                    op=ALU.mult)
            mm(pBA[:], h_sb[:, c, :], pr2[:], start=first, stop=False)
            first = False
            pr3 = chunk_t("pr3")
            nc.vector.tensor_mul(pr3[:], q_c[:], k_b[:, c, :])
            mm(pBA[:], negh_sb[:, c, :], pr3[:],
               start=False, stop=(c == FC - 1))
        p_ba = act.tile([H, NP], BF16, tag="p_ba", bufs=2, name="p_ba")
        nc.scalar.activation(p_ba[:], pBA[:], AF.Sigmoid, scale=ISQ)

        # ---- attention outputs ----
        o_a = unit("F", "o_a")
        o_b = unit("G", "o_b")
        for c in range(FC):
            dv = chunk_t("dv")
            nc.vector.tensor_tensor(_gk(dv[:]), _gk(v_b[:, c, :]),
                                    mview(m_v, c), op=ALU.subtract)
            pBC = ps_mm()
            mm(pBC[:], ht_sb[:, c, :], p_ab[:], start=True, stop=True)
            nc.vector.tensor_mul(o_a[:, c, :], pBC[:], dv[:])
            nc.vector.tensor_tensor(_gk(o_a[:, c, :]), _gk(o_a[:, c, :]),
                                    mview(m_v, c), op=ALU.add)
            pBC2 = ps_mm()
            mm(pBC2[:], ht_sb[:, c, :], p_ba[:], start=True, stop=True)
            nc.vector.tensor_mul(o_b[:, c, :], pBC2[:], dv[:])
            nc.vector.tensor_tensor(o_b[:, c, :], v_b[:, c, :], o_b[:, c, :],
                                    op=ALU.subtract)

        # ---- wo + residual ----
        r_a = unit("hh", "r_a", bufs=2)
        r_b = unit("hh", "r_b", bufs=2)
        for oc in range(FC):
            st = load_strip(t["wo_bf"], oc)
            pA = ps_mm()
            for ic in range(FC):
                mm(pA[:], st[:, ic, :], o_a[:, ic, :],
                   start=(ic == 0), stop=(ic == FC - 1))
            nc.vector.tensor_tensor(_gk(r_a[:, oc, :]), _gk(pA[:]),
                                    mview(m_T, oc), op=ALU.add)
            pB = ps_mm()
            for ic in range(FC):
                mm(pB[:], st[:, ic, :], o_b[:, ic, :],
                   start=(ic == 0), stop=(ic == FC - 1))
            nc.vector.tensor_tensor(r_b[:, oc, :], pB[:], candT[:, oc, :],
                                    op=ALU.add)

        # ---- LN1 (general gains) -> x1 ----
        def layernorm1(r_t, x1_t, tok):
            pSt = ps_stat()
            for c in range(FC):
                sq = chunk_t("sq")
                nc.scalar.activation(sq[:], r_t[:, c, :], AF.Square,
                                     bias=bo_sb[:, c:c + 1])
                mm(pSt[0:1, :], ones_sb[:], r_t[:, c, :],
                   start=(c == 0), stop=(c == FC - 1))
                mm(pSt[32:33, :], ones_sb[:], sq[:],
                   start=(c == 0), stop=(c == FC - 1))
            mu = lane_t("mu" + tok)
            nc.vector.tensor_scalar(mu[:], pSt[0:1, :], s_bo, 1.0 / D,
                                    op0=ALU.add, op1=ALU.mult)
            var = lane_t("var" + tok)
            nc.vector.tensor_mul(var[:], mu[:], mu[:])
            nc.vector.scalar_tensor_tensor(var[:], pSt[32:33, :], 1.0 / D,
                                           var[:], op0=ALU.mult,
                                           op1=ALU.subtract)
            rstd = lane_t("rstd" + tok)
            nc.vector.tensor_scalar_add(var[:], var[:], EPS_LN)
            nc.scalar.activation(rstd[:], var[:], AF.Sqrt)
            nc.vector.reciprocal(rstd[:], rstd[:])
            mubf = act.tile([1, NP], BF16, tag="mubf", bufs=2, name="mubf")
            rstdbf = act.tile([1, NP], BF16, tag="rstdbf", bufs=2,
                              name="rstdbf")
            nc.vector.tensor_copy(mubf[:], mu[:])
            nc.vector.tensor_copy(rstdbf[:], rstd[:])
            mu_bc = ps_mm()
            rstd_bc = ps_mm()
            mm(mu_bc[:], ones_row[:], mubf[:], start=True, stop=True)
            mm(rstd_bc[:], ones_row[:], rstdbf[:], start=True, stop=True)
            for c in range(FC):
                nc.vector.tensor_tensor(x1_t[:, c, :], r_t[:, c, :],
                                        mu_bc[:], op=ALU.subtract)
                nc.vector.scalar_tensor_tensor(
                    x1_t[:, c, :], x1_t[:, c, :], bo_sb[:, c:c + 1],
                    rstd_bc[:], op0=ALU.add, op1=ALU.mult)
                nc.vector.tensor_scalar(
                    x1_t[:, c, :], x1_t[:, c, :], l1g_sb[:, c:c + 1],
                    l1b_sb[:, c:c + 1], op0=ALU.mult, op1=ALU.add)

        x1_a = unit("A", "x1_a")
        x1_b = unit("B", "x1_b")
        layernorm1(r_a, x1_a, "a")
        layernorm1(r_b, x1_b, "b")

        # ---- FFN (both tokens share each weight strip) ----
        h_a = act.tile([P, HFC, NP], BF16, tag="h", bufs=1, name="h_a")
        # token-b hidden aliases four unit tags that are dead by now
        hb = [unit("candT", "hb0"), unit("G", "hb1"),
              unit("F", "hb2"), unit("hh", "hb3", bufs=2)]

        def ha_c(hc):
            return h_a[:, hc, :]

        def hb_c(hc):
            return hb[hc // FC][:, hc % FC, :]

        for hc in range(HFC):
            st = load_strip(t["fw1_bf"], hc)
            for x1_t, hcs in ((x1_a, ha_c), (x1_b, hb_c)):
                pA = ps_mm()
                for ic in range(FC):
                    mm(pA[:], st[:, ic, :], x1_t[:, ic, :],
                       start=(ic == 0), stop=(ic == FC - 1))
                nc.scalar.activation(hcs(hc), pA[:],
                                     AF.Relu, bias=fb1_sb[:, hc:hc + 1])
        r2_a = unit("C2", "r2_a")
        r2_b = unit("D", "r2_b")
        for oc in range(FC):
            stw = wts.tile([P, HFC, P], BF16, tag="w2strip", bufs=2,
                           name="stw")
            nc.gpsimd.dma_start(
                stw[:],
                t["fw2_bf"][oc].rearrange("p (i q) -> p i q", q=P))
            for x1_t, hcs, r2_t in ((x1_a, ha_c, r2_a), (x1_b, hb_c, r2_b)):
                pA = ps_mm()
                for hc in range(HFC):
                    mm(pA[:], stw[:, hc, :], hcs(hc),
                       start=(hc == 0), stop=(hc == HFC - 1))
                nc.vector.tensor_tensor(r2_t[:, oc, :], pA[:],
                                        x1_t[:, oc, :], op=ALU.add)

        # ---- LN2 + cosine via sufficient statistics ----
        def ln2_stats(r2_t, tok):
            pSt = ps_stat()
            for c in range(FC):
                sq = chunk_t("sq")
                nc.scalar.activation(sq[:], r2_t[:, c, :], AF.Square,
                                     bias=fb2_sb[:, c:c + 1])
                mm(pSt[0:1, :], sl3_sb[:, c, 0:1], r2_t[:, c, :],
                   start=(c == 0), stop=(c == FC - 1))
                mm(pSt[32:33, :], sl3_sb[:, c, 1:2], r2_t[:, c, :],
                   start=(c == 0), stop=(c == FC - 1))
                mm(pSt[64:65, :], sl3_sb[:, c, 2:3], r2_t[:, c, :],
                   start=(c == 0), stop=(c == FC - 1))
                mm(pSt[96:97, :], sl3_sb[:, c, 0:1], sq[:],
                   start=(c == 0), stop=(c == FC - 1),
                   tile_position=(0, 96))
            pS2 = ps_stat()
            for c in range(FC):
                sq2 = chunk_t("sq2")
                nc.scalar.activation(sq2[:], r2_t[:, c, :], AF.Square,
                                     bias=fb2_sb[:, c:c + 1])
                mm(pS2[0:1, :], sl3_sb[:, c, 1:2], sq2[:],
                   start=(c == 0), stop=(c == FC - 1))
            # evict the five stats rows into base-0 lane tiles, folding the
            # constant fb2 corrections
            sz = lane_t("sz" + tok)
            nc.vector.tensor_scalar_add(sz[:], pSt[0:1, :], s_fb2)
            g2z = lane_t("g2z" + tok)
            nc.vector.tensor_scalar_add(g2z[:], pSt[32:33, :], s_g2f)
            gbz = lane_t("gbz" + tok)
            nc.vector.tensor_scalar_add(gbz[:], pSt[64:65, :], s_gbf)
            sq_s = lane_t("sq" + tok)
            nc.vector.tensor_copy(sq_s[:], pSt[96:97, :])
            g2q = lane_t("g2q" + tok)
            nc.vector.tensor_copy(g2q[:], pS2[0:1, :])
            return sz, g2z, gbz, sq_s, g2q

        stats_a = ln2_stats(r2_a, "a")
        stats_b = ln2_stats(r2_b, "b")
        pX = ps_head()
        for c in range(FC):
            rr = chunk_t("rr")
            nc.vector.tensor_scalar_add(rr[:], r2_b[:, c, :],
                                        fb2_sb[:, c:c + 1])
            nc.vector.scalar_tensor_tensor(rr[:], r2_a[:, c, :],
                                           fb2_sb[:, c:c + 1], rr[:],
                                           op0=ALU.add, op1=ALU.mult)
            mm(pX[:], sl3_sb[:, c, 1:2], rr[:],
               start=(c == 0), stop=(c == FC - 1))

        # lane algebra for cosine
        def ln2_lane(stats, tok):
            sz, g2z, gbz, sq_s, g2q = stats
            muz = lane_t("muz" + tok)
            nc.vector.tensor_scalar_mul(muz[:], sz[:], 1.0 / D)
            var = lane_t("var2" + tok)
            nc.vector.tensor_mul(var[:], muz[:], muz[:])
            nc.vector.scalar_tensor_tensor(var[:], sq_s[:], 1.0 / D,
                                           var[:], op0=ALU.mult,
                                           op1=ALU.subtract)
            rstd = lane_t("rstd2" + tok)
            nc.vector.tensor_scalar_add(var[:], var[:], EPS_LN)
            nc.scalar.activation(rstd[:], var[:], AF.Sqrt)
            nc.vector.reciprocal(rstd[:], rstd[:])
            return muz, rstd, g2z, gbz, g2q

        mua, rsta, g2za, gbza, g2qa = ln2_lane(stats_a, "a")
        mub2, rstb, g2zb, gbzb, g2qb = ln2_lane(stats_b, "b")

        def gbt(mu, rstd, gbz, name):
            o_t = lane_t(name)
            nc.vector.tensor_scalar_mul(o_t[:], mu[:], s_gb)
            nc.vector.tensor_tensor(o_t[:], gbz[:], o_t[:], op=ALU.subtract)
            nc.vector.tensor_mul(o_t[:], o_t[:], rstd[:])
            return o_t

        gbta = gbt(mua, rsta, gbza, "gbta")
        gbtb = gbt(mub2, rstb, gbzb, "gbtb")

        def normsq(mu, rstd, g2z, g2q, gbt_t, name):
            o_t = lane_t(name)
            nc.vector.tensor_scalar_mul(o_t[:], mu[:], s_g2)
            nc.vector.scalar_tensor_tensor(o_t[:], g2z[:], -2.0, o_t[:],
                                           op0=ALU.mult, op1=ALU.add)
            nc.vector.tensor_mul(o_t[:], o_t[:], mu[:])
            nc.vector.tensor_add(o_t[:], o_t[:], g2q[:])
            nc.vector.tensor_mul(o_t[:], o_t[:], rstd[:])
            nc.vector.tensor_mul(o_t[:], o_t[:], rstd[:])
            nc.vector.scalar_tensor_tensor(o_t[:], gbt_t[:], 2.0, o_t[:],
                                           op0=ALU.mult, op1=ALU.add)
            nc.vector.tensor_scalar_add(o_t[:], o_t[:], s_bb)
            return o_t

        n2a = normsq(mua, rsta, g2za, g2qa, gbta, "n2a")
        n2b = normsq(mub2, rstb, g2zb, g2qb, gbtb, "n2b")

        d01 = lane_t("d01")
        nc.vector.tensor_scalar_mul(d01[:], mub2[:], s_g2)
        nc.vector.tensor_tensor(d01[:], d01[:], g2zb[:], op=ALU.subtract)
        nc.vector.tensor_mul(d01[:], d01[:], mua[:])
        t2 = lane_t("t2")
        nc.vector.tensor_mul(t2[:], mub2[:], g2za[:])
        nc.vector.tensor_tensor(d01[:], d01[:], t2[:], op=ALU.subtract)
        nc.vector.tensor_tensor(d01[:], pX[:], d01[:], op=ALU.add)
        nc.vector.tensor_mul(d01[:], d01[:], rsta[:])
        nc.vector.tensor_mul(d01[:], d01[:], rstb[:])
        nc.vector.tensor_add(d01[:], d01[:], gbta[:])
        nc.vector.tensor_add(d01[:], d01[:], gbtb[:])
        nc.vector.tensor_scalar_add(d01[:], d01[:], s_bb)

        den = lane_t("den")
        nc.scalar.activation(n2a[:], n2a[:], AF.Sqrt)
        nc.vector.tensor_scalar_max(n2a[:], n2a[:], EPS_COS)
        nc.scalar.activation(n2b[:], n2b[:], AF.Sqrt)
        nc.vector.tensor_scalar_max(n2b[:], n2b[:], EPS_COS)
        nc.vector.tensor_mul(den[:], n2a[:], n2b[:])
        nc.vector.reciprocal(den[:], den[:])
        atg_sl = lane_t("atg_sl")
        nc.vector.tensor_mul(atg_sl[:], d01[:], den[:])
        nc.gpsimd.dma_start(t["out"][1:2, ts(mt, NP)], atg_sl[:])


# ===================== host side =====================

def kernel(**inputs):
    f32 = np.float32
    bf16 = ml_dtypes.bfloat16
    txt = np.ascontiguousarray(
        np.asarray(inputs["text_embeddings"], f32).reshape(S, D))
    cand_full = np.ascontiguousarray(
        np.asarray(inputs["candidate_embeddings"], f32).reshape(M * K, D))
    starts = np.asarray(inputs["mention_starts"], np.int64)
    spans = np.asarray(inputs["span_lengths"], np.int64)
    ends = starts + spans

    j = np.stack([ends + 1, starts,
                  np.minimum(S - 1, ends + CTX),
                  np.maximum(0, starts - CTX)], axis=1)       # [M, 4]
    chunk_of = (np.maximum(j - 1, 0) // P).astype(np.int64)   # [M, 4]
    inv = np.stack([1.0 / (spans + 1).astype(f32),
                    1.0 / (j[:, 2] - j[:, 3]).astype(f32)], axis=1)

    consts = {
        "tri": np.triu(np.ones((P, P), f32)),
        "tri32": np.triu(np.ones((NCH, NCH), f32), k=1),
        "ident": np.eye(P, dtype=f32),
        "identb": np.eye(P, dtype=f32).astype(bf16),
        "zrow": np.zeros((1, D), f32),
        "hmat": np.repeat(np.eye(H, dtype=f32), DH, axis=0).astype(bf16),
        "i8neg": (-np.eye(H, dtype=f32)).astype(bf16),
    }
    wnames = ["relik_w1", "relik_b1", "relik_w2",
              "wq", "bq", "wk", "bk", "wv", "bv", "wo", "bo",
              "ln1_g", "ln1_b", "ffn_w1", "ffn_b1", "ffn_w2", "ffn_b2",
              "ln2_g", "ln2_b", "uni_w1", "uni_b1", "uni_w2"]
    weights = {n: np.ascontiguousarray(np.asarray(inputs[n], f32))
               for n in wnames}
    weights["relik_b2"] = np.asarray(inputs["relik_b2"], f32).reshape(1, 1)
    weights["uni_b2"] = np.ascontiguousarray(
        np.asarray(inputs["uni_b2"], f32).reshape(1, D))

    in_maps = []
    for core in range(NCORES):
        sl = slice(core * M_LOC, (core + 1) * M_LOC)
        selt = np.zeros((NCH, 2, P), f32)
        jc = chunk_of[sl]                                     # [128, 4]
        ar = np.arange(P)
        for col, (tp, tm) in enumerate(((0, 1), (2, 3))):
            np.add.at(selt, (jc[:, tp], col, ar), 1.0)
            np.add.at(selt, (jc[:, tm], col, ar), -1.0)
        im = {
            "txt": txt,
            "cand": cand_full[core * PAIRS:(core + 1) * PAIRS],
            "idx": np.ascontiguousarray(j[sl].astype(np.int32)),
            "invl": np.ascontiguousarray(inv[sl].astype(f32)),
            "seltab": selt,
        }
        im.update(consts)
        im.update(weights)
        in_maps.append(im)

    if "nc" not in _NC_CACHE:
        _NC_CACHE["nc"] = _build_nc()
    nc = _NC_CACHE["nc"]

    results = bass_utils.run_bass_kernel_spmd(
        nc, in_maps, core_ids=list(range(NCORES))).results

    out = np.zeros((3, M, K), f32)
    for core in range(NCORES):
        sl = slice(core * M_LOC, (core + 1) * M_LOC)
        out[:, sl, :] = results[core]["out"].reshape(3, M_LOC, K)
    return out


if __name__ == "__main__":
    nc = _build_nc()
    print("built ok")



# revision 29
# speedup vs baseline: 1.8529x; 1.8529x over previous
"""Trainium2 Bass kernel for nn_EntityResolutionProcessor.

Strategy: data-parallel over mentions (M=1024 -> 128/core on 8 cores).
v2: fp8e4 DoubleRow matmuls (x64 weight scale) for qkv/wo/FFN/uni paths
(relik stays bf16 for accuracy); weights SBUF-resident, quantized host-side
and uploaded in fp8/bf16; mention/context means precomputed on host
(cumsum+gather); candidates uploaded bf16; PSUM-evict work spread across
ACT/DVE/Pool engines; biases folded into PSUM via bias-row matmuls where
the evict engine lacks them.
"""

from contextlib import ExitStack

import ml_dtypes
import numpy as np

import concourse.bass as bass
import concourse.mybir as mybir
import concourse.tile as tile
from concourse import bacc, bass_isa, bass_utils
from concourse.bass import ds, ts

S, D, M, K, H = 4096, 768, 1024, 32, 8
DH = D // H
CTX = 10
NCORES = 8
P = 128
FC = D // P                     # 6 feature chunks
HFC = 4 * D // P                # 24 ffn hidden chunks
M_LOC = M // NCORES             # 128 mentions per core
PAIRS = M_LOC * K               # 4096 pairs per core
NP = 512                        # pairs per macro tile
G = NP // K                     # 16 mentions per macro tile
NMACRO = PAIRS // NP            # 8
ISQ = 1.0 / float(np.sqrt(np.float32(DH)))
EPS_LN = 1e-5
EPS_COS = 1e-8
WS = 64.0                       # fp8 weight scale
IWS = 1.0 / WS

F32 = mybir.dt.float32
BF16 = mybir.dt.bfloat16
FP8 = mybir.dt.float8e4
AF = mybir.ActivationFunctionType
ALU = mybir.AluOpType
DR = mybir.MatmulPerfMode.DoubleRow

_NC_CACHE = {}


def _gk(ap):
    """view a [128, NP] AP as [128, G, K]"""
    return ap.rearrange("p (g k) -> p g k", g=G)


def _build_nc():
    nc = bacc.Bacc(
        "TRN2", target_bir_lowering=False, debug=False, num_devices=NCORES
    )

    def inp(name, shape, dtype=F32):
        return nc.dram_tensor(name, list(shape), dtype, kind="ExternalInput").ap()

    t = {}
    t["cand"] = inp("cand", [PAIRS, D], BF16)
    t["mrow"] = inp("mrow", [P, D])        # mention means (row-major)
    t["crow"] = inp("crow", [P, D])        # context means
    t["ident"] = inp("ident", [P, P])
    t["identb"] = inp("identb", [P, P], BF16)
    t["hmat"] = inp("hmat", [D, H], BF16)  # head indicator
    t["i8neg"] = inp("i8neg", [H, H], BF16)
    t["idhfc"] = inp("idhfc", [HFC, HFC], BF16)
    # bias rows (x64 for psum folds), bf16 [1, X]
    t["borow64"] = inp("borow64", [FC, P], BF16)
    t["fb1row64"] = inp("fb1row64", [HFC, P], BF16)
    t["fb2row64"] = inp("fb2row64", [FC, P], BF16)
    # pre-quantized weights (host side): [P, noc, nic*P]
    t["wq8"] = inp("wq8", [P, FC, D], FP8)
    t["wk8"] = inp("wk8", [P, FC, D], FP8)
    t["wv8"] = inp("wv8", [P, FC, D], FP8)
    t["wo8"] = inp("wo8", [P, FC, D], FP8)
    t["u1b8"] = inp("u1b8", [P, FC, D], FP8)
    t["fw18"] = inp("fw18", [P, HFC, D], FP8)
    t["fw28"] = inp("fw28", [P, FC, 4 * D], FP8)
    t["w1bb"] = inp("w1bb", [P, FC, D], BF16)
    t["w1ab"] = inp("w1ab", [P, FC, D], BF16)
    t["u2rs64"] = inp("u2rs64", [P, FC], BF16)
    t["b2m"] = inp("b2m", [1, 1])

    t["ffn_b1"] = inp("ffn_b1", [4 * D])
    for n, shp in [("relik_b1", [D]), ("relik_w2", [D, 1]),
                   ("relik_b2", [1, 1]),
                   ("bq", [D]), ("bk", [D]), ("bv", [D]),
                   ("ln1_g", [D]), ("ln1_b", [D]),
                   ("ln2_g", [D]), ("ln2_b", [D]),
                   ("uni_b1", [D])]:
        t[n] = inp(n, shp)

    t["out"] = nc.dram_tensor("out", [3, PAIRS], F32, kind="ExternalOutput").ap()

    with tile.TileContext(nc) as tc:
        _body(nc, tc, t)
    nc.compile()
    return nc


def _body(nc, tc, t):
    with ExitStack() as _ctx:
        _body_inner(nc, tc, t, _ctx)


def _vec6(v_ap, n=FC):
    return v_ap.rearrange("(i p) -> p i", p=P)


def _body_inner(nc, tc, t, _ctx):
    mm = lambda *a, **k: nc.tensor.matmul(*a, **k)

    # ---------------- pools ----------------
    psum = _ctx.enter_context(tc.tile_pool(name="psum", bufs=1, space="PSUM"))
    res = _ctx.enter_context(tc.tile_pool(name="res", bufs=1))

    def ps_mm(shape=(P, NP), dtype=F32):
        return psum.tile(list(shape), dtype, tag="mm", bufs=4,
                         padded_shape=[P, NP], name="ps_mm")

    def ps_score():
        return psum.tile([8, NP], F32, tag="score", bufs=1, name="ps_score")

    def ps_stat():
        # stats tile: MM groups land at base partitions 0/32/64/96
        return psum.tile([P, NP], F32, tag="stat", bufs=2, name="ps_stat")

    def ps_head():
        return psum.tile([1, NP], F32, tag="head", bufs=1, name="ps_head")

    # ---------------- resident constants ----------------
    sp = nc.sync

    def load_res(name, ap_src, shape, dtype=F32, q=sp):
        tl = res.tile(list(shape), dtype, name=name)
        q.dma_start(tl[:], ap_src)
        return tl

    ident_sb = load_res("ident_sb", t["ident"][:], [P, P])
    identb_sb = load_res("identb_sb", t["identb"][:], [P, P], BF16)
    i8neg_sb = load_res("i8neg_sb", t["i8neg"][:], [H, H], BF16)
    h_sb = load_res("h_sb", t["hmat"].rearrange("(c p) h -> p c h", p=P),
                    [P, FC, H], BF16)
    ht_sb = load_res("ht_sb", t["hmat"].rearrange("(c p) h -> h c p", p=P),
                     [H, FC, P], BF16)
    negh_sb = res.tile([P, FC, H], BF16, name="negh_sb")
    nc.vector.tensor_scalar_mul(negh_sb[:], h_sb[:], -1.0)

    # bias rows for psum folds: row c at partition c (lhsT slice [c:c+1])
    borow_sb = load_res("borow_sb", t["borow64"][:], [FC, P], BF16)
    fb1row_sb = load_res("fb1row_sb", t["fb1row64"][:], [HFC, P], BF16)
    fb2row_sb = load_res("fb2row_sb", t["fb2row64"][:], [FC, P], BF16)

    bq_sb = load_res("bq_sb", _vec6(t["bq"]), [P, FC])
    bk_sb = load_res("bk_sb", _vec6(t["bk"]), [P, FC])
    bv_sb = load_res("bv_sb", _vec6(t["bv"]), [P, FC])
    rb1_sb = load_res("rb1_sb", _vec6(t["relik_b1"]), [P, FC])
    ub1_sb = load_res("ub1_sb", _vec6(t["uni_b1"]), [P, FC])
    fb1_sb = load_res("fb1_sb", _vec6(t["ffn_b1"], HFC), [P, HFC])
    l1g_sb = load_res("l1g_sb", _vec6(t["ln1_g"]), [P, FC])
    l1b_sb = load_res("l1b_sb", _vec6(t["ln1_b"]), [P, FC])
    l2g_sb = load_res("l2g_sb", _vec6(t["ln2_g"]), [P, FC])
    l2b_sb = load_res("l2b_sb", _vec6(t["ln2_b"]), [P, FC])
    rw2_sb = load_res("rw2_sb",
                      t["relik_w2"].rearrange("(c p) o -> p c o", p=P),
                      [P, FC, 1], BF16, q=nc.gpsimd)
    rb2_sb = load_res("rb2_sb", t["relik_b2"][:], [1, 1])
    u2rs_sb = load_res("u2rs_sb", t["u2rs64"][:, :, None], [P, FC, 1], BF16)
    b2m_sb = load_res("b2m_sb", t["b2m"][:], [1, 1])

    # resident weights [P, noc, nic, P]; DR lhsT slice [:, oc, 2i:2i+2, :]
    def load_w(name, nic=FC, noc=FC, dtype=FP8):
        tl = res.tile([P, noc, nic, P], dtype, name=name + "_sb")
        sp.dma_start(tl[:], t[name].rearrange("p o (i q) -> p o i q", q=P))
        return tl

    wq8 = load_w("wq8")
    wk8 = load_w("wk8")
    wv8 = load_w("wv8")
    wo8 = load_w("wo8")
    u1b8 = load_w("u1b8")
    fw1_8 = load_w("fw18", noc=HFC)
    fw2_8 = load_w("fw28", nic=HFC)
    w1b_b16 = load_w("w1bb", dtype=BF16)

    ones_sb = res.tile([P, 1], BF16, name="ones_sb")
    nc.vector.memset(ones_sb[:], 1.0)
    onesf_sb = res.tile([P, 1], F32, name="onesf_sb")
    nc.vector.memset(onesf_sb[:], 1.0)
    ones_row = res.tile([1, P], BF16, name="ones_row")
    nc.vector.memset(ones_row[:], 1.0)
    idh_sb = load_res("idh_sb", t["idhfc"][:], [HFC, HFC], BF16)

    def bias_mm(pA, row_sb, r, nrow, n=NP):
        mm(pA[:], row_sb[:],
           idh_sb[:nrow, r, None].to_broadcast([nrow, n]),
           start=False, stop=True)

    # stats lhsT [128, 6, 3]: cols = [1, g2^2, g2*b2] per feature chunk
    sl3_sb = res.tile([P, FC, 3], BF16, name="sl3_sb")
    g2sq_sb = res.tile([P, FC], F32, name="g2sq_sb")
    g2b2_sb = res.tile([P, FC], F32, name="g2b2_sb")
    nc.vector.tensor_mul(g2sq_sb[:], l2g_sb[:], l2g_sb[:])
    nc.vector.tensor_mul(g2b2_sb[:], l2g_sb[:], l2b_sb[:])
    for c in range(FC):
        nc.vector.tensor_copy(sl3_sb[:, c, 0:1], ones_sb[:])
        nc.vector.tensor_copy(sl3_sb[:, c, 1:2], g2sq_sb[:, c:c + 1])
        nc.vector.tensor_copy(sl3_sb[:, c, 2:3], g2b2_sb[:, c:c + 1])
        pass

    # scalar reductions of gain/bias vectors (each -> [1,1] on partition 0)
    def vec_sum(name, vecs):
        tmp = res.tile([P, FC], F32, name=name + "_t")
        if len(vecs) == 1:
            nc.vector.tensor_copy(tmp[:], vecs[0][:])
        else:
            nc.vector.tensor_mul(tmp[:], vecs[0][:], vecs[1][:])
            for v in vecs[2:]:
                nc.vector.tensor_mul(tmp[:], tmp[:], v[:])
        red = res.tile([P, 1], F32, name=name + "_r")
        nc.vector.tensor_reduce(red[:], tmp[:], axis=mybir.AxisListType.X,
                                op=ALU.add)
        pR = ps_head()
        mm(pR[:, 0:1], red[:], onesf_sb[:], start=True, stop=True)
        arr = res.tile([1, 1], F32, name=name)
        nc.vector.tensor_copy(arr[:], pR[:, 0:1])
        return arr[0:1, 0:1]

    s_g2 = vec_sum("s_g2", [l2g_sb, l2g_sb])
    s_gb = vec_sum("s_gb", [l2g_sb, l2b_sb])
    s_bb = vec_sum("s_bb", [l2b_sb, l2b_sb])

    # per-mention outputs (feature-major [128, 6, 128])
    womv = res.tile([P, FC, P], BF16, name="womv")   # wo@m_v + m_T + bo
    m_Tb = res.tile([P, FC, P], BF16, name="m_Tb")   # bf16: relik rhs
    m_T8 = res.tile([P, FC, P], FP8, name="m_T8")    # fp8: DR rhs
    c_T8 = res.tile([P, FC, P], FP8, name="c_T8")
    m_q = res.tile([P, FC, P], BF16, name="m_q")
    m_k = res.tile([P, FC, P], BF16, name="m_k")
    m_v = res.tile([P, FC, P], BF16, name="m_v")
    m_relik = res.tile([P, FC, P], BF16, name="m_relik")
    c_uni = res.tile([P, FC, P], BF16, name="c_uni")
    s_aa_sb = res.tile([H, P], BF16, name="s_aa_sb")

    # ================= phase 0: transposes + per-mention =================
    with tc.tile_pool(name="p0", bufs=1) as p0:
        mrow = p0.tile([P, D], F32, name="mrow_t")
        crow = p0.tile([P, D], F32, name="crow_t")
        sp.dma_start(mrow[:], t["mrow"][:])
        sp.dma_start(crow[:], t["crow"][:])
        m_T = p0.tile([P, FC, P], F32, name="m_T")
        for fc in range(FC):
            pT = ps_mm((P, P))
            nc.tensor.transpose(pT[:], mrow[:, ts(fc, P)], ident_sb[:])
            nc.vector.tensor_copy(m_T[:, fc, :], pT[:])
            nc.any.tensor_copy(m_Tb[:, fc, :], pT[:])
            nc.scalar.activation(m_T8[:, fc, :], pT[:], AF.Copy)
            pT2 = ps_mm((P, P))
            nc.tensor.transpose(pT2[:], crow[:, ts(fc, P)], ident_sb[:])
            nc.scalar.activation(c_T8[:, fc, :], pT2[:], AF.Copy)

        # per-mention projections: q/k/v/uni via fp8 DR; relik via bf16
        for w8, b_sb, out_t, src8 in (
            (wq8, bq_sb, m_q, m_T8),
            (wk8, bk_sb, m_k, m_T8),
            (wv8, bv_sb, m_v, m_T8),
            (u1b8, ub1_sb, c_uni, c_T8),
        ):
            for oc in range(FC):
                pA = ps_mm((P, P))
                for i in range(FC // 2):
                    mm(pA[:], w8[:, oc, 2 * i:2 * i + 2, :],
                       src8[:, 2 * i:2 * i + 2, :],
                       start=(i == 0), stop=(i == FC // 2 - 1), perf_mode=DR)
                nc.scalar.activation(out_t[:, oc, :], pA[:], AF.Identity,
                                     bias=b_sb[:, oc:oc + 1], scale=IWS)
        m_v8 = p0.tile([P, FC, P], FP8, name="m_v8")
        for c in range(FC):
            nc.scalar.activation(m_v8[:, c, :], m_v[:, c, :], AF.Copy)
        for oc in range(FC):
            pA = ps_mm((P, P))
            for i in range(FC // 2):
                mm(pA[:], wo8[:, oc, 2 * i:2 * i + 2, :],
                   m_v8[:, 2 * i:2 * i + 2, :],
                   start=(i == 0), stop=False, perf_mode=DR)
            bias_mm(pA, borow_sb, oc, FC, n=P)
            nc.vector.scalar_tensor_tensor(
                womv[:, oc, :], pA[:], IWS, m_T[:, oc, :],
                op0=ALU.mult, op1=ALU.add)
        w1a_st = p0.tile([P, FC, FC, P], BF16, name="w1a_st")
        sp.dma_start(w1a_st[:],
                     t["w1ab"].rearrange("p o (i q) -> p o i q", q=P))
        for oc in range(FC):
            pA = ps_mm((P, P))
            for ic in range(FC):
                mm(pA[:], w1a_st[:, oc, ic, :], m_Tb[:, ic, :],
                   start=(ic == 0), stop=(ic == FC - 1))
            nc.scalar.activation(m_relik[:, oc, :], pA[:], AF.Identity,
                                 bias=rb1_sb[:, oc:oc + 1])

        # s_aa [8, 128]
        mprod = p0.tile([P, FC, P], BF16, name="mprod")
        for c in range(FC):
            nc.vector.tensor_mul(mprod[:, c, :], m_q[:, c, :], m_k[:, c, :])
        pS = ps_score()
        for c in range(FC):
            mm(pS[:, :P], h_sb[:, c, :], mprod[:, c, :],
               start=(c == 0), stop=(c == FC - 1))
        nc.any.tensor_copy(s_aa_sb[:], pS[:, :P])

    # ================= pools for the main phase =================
    act = _ctx.enter_context(tc.tile_pool(name="act", bufs=1))
    lane = _ctx.enter_context(tc.tile_pool(name="lane", bufs=1))

    def unit(tag, name, dtype=BF16, bufs=1):
        return act.tile([P, FC, NP], dtype, tag=tag, bufs=bufs, name=name)

    def chunk_t(name, dtype=BF16):
        return act.tile([P, NP], dtype, tag="tt", bufs=3, name=name)

    def dr6(pA, w8, oc, rhs8, extra=None):
        """3 DoubleRow matmuls contracting 6 feature chunks (+ optional
        bias-row matmul appended to the accumulation group)."""
        n = FC // 2
        for i in range(n):
            mm(pA[:], w8[:, oc, 2 * i:2 * i + 2, :],
               rhs8[:, 2 * i:2 * i + 2, :],
               start=(i == 0), stop=(i == n - 1 and extra is None),
               perf_mode=DR)
        if extra is not None:
            row_sb, r, nrow = extra
            bias_mm(pA, row_sb, r, nrow)

    # ============ macro-tile loop (2-deep software pipeline) ============
    def lane_t(name):
        return lane.tile([1, NP], F32, tag="lt", bufs=12, name=name)

    def mview_of(mt):
        gsl = ds(mt * G, G)

        def mview(mt_tile, c):
            return mt_tile[:, c, gsl, None].to_broadcast([P, G, K])
        return gsl, mview

    def prepare(mt):
        """candidate DMA + transpose + k/v projections for tile mt."""
        cand_rm = act.tile([P, 4, D], BF16, tag="cand_rm", bufs=1,
                           name="cand_rm")
        sp.dma_start(
            cand_rm[:],
            t["cand"].rearrange("(q p) d -> p q d", p=P)[:, ds(4 * mt, 4), :])
        candT = unit("candT", "candT")
        candT8 = unit("candT8", "candT8", FP8)
        for fc in range(FC):
            pT = ps_mm(dtype=BF16)
            for pc in range(4):
                nc.tensor.transpose(pT[:, ts(pc, P)],
                                    cand_rm[:, pc, ts(fc, P)], identb_sb[:])
            nc.scalar.activation(candT[:, fc, :], pT[:], AF.Copy)
            nc.gpsimd.tensor_copy(candT8[:, fc, :], candT[:, fc, :])
        k_b = unit("B", "k_b")
        v_b = unit("C", "v_b")
        for w8, b_sb, out_t in ((wk8, bk_sb, k_b), (wv8, bv_sb, v_b)):
            for oc in range(FC):
                pA = ps_mm()
                dr6(pA, w8, oc, candT8)
                nc.scalar.activation(out_t[:, oc, :], pA[:], AF.Identity,
                                     bias=b_sb[:, oc:oc + 1], scale=IWS)
        return candT, candT8, k_b, v_b

    def a_stage(mt, prep):
        """relik/uni heads, attention scores+outputs, wo residual."""
        candT, candT8, k_b, v_b = prep
        gsl, mview = mview_of(mt)

        # relik head (bf16)
        h_r = unit("hh", "h_r", bufs=2)
        for oc in range(FC):
            pA = ps_mm()
            for ic in range(FC):
                mm(pA[:], w1b_b16[:, oc, ic, :], candT[:, ic, :],
                   start=(ic == 0), stop=(ic == FC - 1))
            nc.vector.tensor_tensor(_gk(h_r[:, oc, :]), _gk(pA[:]),
                                    mview(m_relik, oc), op=ALU.add)
            nc.scalar.activation(h_r[:, oc, :], h_r[:, oc, :], AF.Relu)
        pH = ps_head()
        for c in range(FC):
            mm(pH[:], rw2_sb[:, c, :], h_r[:, c, :],
               start=(c == 0), stop=(c == FC - 1))
        osl_r = lane_t("osl_r")
        nc.scalar.activation(osl_r[:], pH[:], AF.Identity, bias=rb2_sb[:])
        sp.dma_start(t["out"][0:1, ts(mt, NP)], osl_r[:])

        # uni head (fp8 DR)
        h_u = unit("hu8", "h_u", FP8)
        for oc in range(FC):
            pA = ps_mm()
            dr6(pA, u1b8, oc, candT8)
            hp = chunk_t("hp")
            nc.vector.scalar_tensor_tensor(
                _gk(hp[:]), _gk(pA[:]), IWS, mview(c_uni, oc),
                op0=ALU.mult, op1=ALU.add)
            nc.scalar.activation(h_u[:, oc, :], hp[:], AF.Relu)
        pH2 = ps_head()
        for c in range(FC):
            mm(pH2[:], u2rs_sb[:, c, :], h_u[:, c, :],
               start=(c == 0), stop=(c == FC - 1))
        osl_u = lane_t("osl_u")
        nc.scalar.activation(osl_u[:], pH2[:], AF.Sigmoid, bias=b2m_sb[:],
                             scale=1.0 / (WS * D))
        sp.dma_start(t["out"][2:3, ts(mt, NP)], osl_u[:])

        # attention scores A->B
        pAB = ps_score()
        for c in range(FC):
            pr1 = chunk_t("pr1")
            nc.gpsimd.tensor_tensor(_gk(pr1[:]), _gk(k_b[:, c, :]),
                                    mview(m_q, c), op=ALU.mult)
            mm(pAB[:], h_sb[:, c, :], pr1[:], start=(c == 0), stop=False)
        mm(pAB[:], i8neg_sb[:],
           s_aa_sb[:, gsl, None].to_broadcast([H, G, K]),
           start=False, stop=True)
        p_ab = act.tile([H, NP], BF16, tag="p_ab", bufs=2, name="p_ab")
        nc.scalar.activation(p_ab[:], pAB[:], AF.Sigmoid, scale=ISQ)

        # attention scores B->A (q proj pipelined one chunk ahead)
        pBA = psum.tile([8, NP], F32, tag="stat", bufs=2,
                        padded_shape=[P, NP], name="pBA")
        qs = []
        first = [True]

        def ba_reduce(c, q_c):
            pr2 = act.tile([P, NP], BF16, tag="pr", bufs=2, name="pr2")
            nc.gpsimd.tensor_tensor(_gk(pr2[:]), _gk(q_c[:]), mview(m_k, c),
                                    op=ALU.mult)
            mm(pBA[:], h_sb[:, c, :], pr2[:], start=first[0], stop=False)
            first[0] = False
            pr3 = act.tile([P, NP], BF16, tag="pr", bufs=2, name="pr3")
            nc.vector.tensor_mul(pr3[:], q_c[:], k_b[:, c, :])
            mm(pBA[:], negh_sb[:, c, :], pr3[:],
               start=False, stop=(c == FC - 1))

        for c in range(FC):
            pQ = ps_mm()
            dr6(pQ, wq8, c, candT8)
            q_c = chunk_t("q_c")
            nc.scalar.activation(q_c[:], pQ[:], AF.Identity,
                                 bias=bq_sb[:, c:c + 1], scale=IWS)
            qs.append(q_c)
            if c >= 1:
                ba_reduce(c - 1, qs[c - 1])
        ba_reduce(FC - 1, qs[FC - 1])
        p_ba = act.tile([H, NP], BF16, tag="p_ba", bufs=2, name="p_ba")
        nc.scalar.activation(p_ba[:], pBA[:], AF.Sigmoid, scale=ISQ)

        # attention outputs (fp8 for wo rhs)
        o_a = unit("F8", "o_a", FP8)
        o_b = unit("G8", "o_b", FP8)
        for c in range(FC):
            dv = chunk_t("dv")
            nc.gpsimd.tensor_tensor(_gk(dv[:]), _gk(v_b[:, c, :]),
                                    mview(m_v, c), op=ALU.subtract)
            pBC = ps_mm()
            mm(pBC[:], ht_sb[:, c, :], p_ab[:], start=True, stop=True)
            nc.vector.tensor_mul(o_a[:, c, :], pBC[:], dv[:])
            pBC2 = ps_mm()
            mm(pBC2[:], ht_sb[:, c, :], p_ba[:], start=True, stop=True)
            pdv2 = chunk_t("pdv2")
            nc.vector.tensor_mul(pdv2[:], pBC2[:], dv[:])
            nc.gpsimd.tensor_tensor(o_b[:, c, :], v_b[:, c, :], pdv2[:],
                                    op=ALU.subtract)

        # wo + residual
        r_a = unit("hh", "r_a", bufs=2)
        r_b = unit("rb", "r_b")
        for oc in range(FC):
            pA = ps_mm()
            dr6(pA, wo8, oc, o_a)
            nc.vector.scalar_tensor_tensor(
                _gk(r_a[:, oc, :]), _gk(pA[:]), IWS, mview(womv, oc),
                op0=ALU.mult, op1=ALU.add)
            pB = ps_mm()
            dr6(pB, wo8, oc, o_b, extra=(borow_sb, oc, FC))
            nc.vector.scalar_tensor_tensor(
                r_b[:, oc, :], pB[:], IWS, candT[:, oc, :],
                op0=ALU.mult, op1=ALU.add)
        return candT, r_a, r_b

    def ln1_block(ar):
        candT, r_a, r_b = ar
        x1_a = unit("A8", "x1_a", FP8)
        x1_b = unit("B8", "x1_b", FP8)
        ln1 = []
        for r_t, tok in ((r_a, "a"), (r_b, "b")):
            pSt = ps_stat()
            for c in range(FC):
                sq = chunk_t("sq")
                nc.gpsimd.tensor_mul(sq[:], r_t[:, c, :], r_t[:, c, :])
                mm(pSt[0:1, :], ones_sb[:], r_t[:, c, :],
                   start=(c == 0), stop=(c == FC - 1))
                mm(pSt[32:33, :], ones_sb[:], sq[:],
                   start=(c == 0), stop=(c == FC - 1))
            ln1.append(pSt)
        mr = []
        for pSt, tok in zip(ln1, "ab"):
            mu = lane_t("mu" + tok)
            nc.vector.tensor_scalar_mul(mu[:], pSt[0:1, :], 1.0 / D)
            var = lane_t("var" + tok)
            nc.vector.tensor_mul(var[:], mu[:], mu[:])
            nc.vector.scalar_tensor_tensor(var[:], pSt[32:33, :], 1.0 / D,
                                           var[:], op0=ALU.mult,
                                           op1=ALU.subtract)
            rstd = lane_t("rstd" + tok)
            nc.vector.tensor_scalar_add(var[:], var[:], EPS_LN)
            nc.scalar.activation(rstd[:], var[:], AF.Sqrt)
            nc.vector.reciprocal(rstd[:], rstd[:])
            mubf = act.tile([1, NP], BF16, tag="mubf", bufs=1, name="mubf")
            rstdbf = act.tile([1, NP], BF16, tag="rstdbf", bufs=1,
                              name="rstdbf")
            nc.scalar.activation(mubf[:], mu[:], AF.Copy)
            nc.scalar.activation(rstdbf[:], rstd[:], AF.Copy)
            mu_sb = act.tile([P, NP], BF16, tag="mu_sb" + tok, bufs=1,
                             name="mu_sb")
            rstd_sb = act.tile([P, NP], BF16, tag="rstd_sb" + tok, bufs=1,
                               name="rstd_sb")
            rstd_bc = ps_mm()
            mm(rstd_bc[:], ones_row[:], rstdbf[:], start=True, stop=True)
            nc.scalar.activation(rstd_sb[:], rstd_bc[:], AF.Copy)
            mu_bc = ps_mm()
            mm(mu_bc[:], ones_row[:], mubf[:], start=True, stop=True)
            nc.scalar.activation(mu_sb[:], mu_bc[:], AF.Copy)
            mr.append((mu_sb, rstd_sb))
        for (r_t, x1_t), (mu_sb, rstd_sb) in zip(
                ((r_a, x1_a), (r_b, x1_b)), mr):
            for c in range(FC):
                t1 = chunk_t("t1")
                nc.gpsimd.tensor_tensor(t1[:], r_t[:, c, :], mu_sb[:],
                                        op=ALU.subtract)
                t2 = chunk_t("t2")
                nc.vector.tensor_mul(t2[:], t1[:], rstd_sb[:])
                nc.vector.tensor_scalar(x1_t[:, c, :], t2[:],
                                        l1g_sb[:, c:c + 1],
                                        l1b_sb[:, c:c + 1],
                                        op0=ALU.mult, op1=ALU.add)
        return x1_a, x1_b

    def ffn1_block(x1s):
        x1_a, x1_b = x1s
        ha = act.tile([P, HFC, NP], FP8, tag="ha8", bufs=1, name="h_a")
        ha = [ha[:, 6 * j:6 * (j + 1), :] for j in range(4)]
        hb = [unit("F8", "hb0", FP8), unit("G8", "hb1", FP8),
              unit("hu8", "hb2", FP8), unit("hh", "hb3", FP8, bufs=2)]
        for x1_t, hts, half in ((x1_a, ha, 0), (x1_b, hb, 1)):
            for hc in range(HFC):
                hout = hts[hc // FC][:, hc % FC, :]
                e = (hc * 2 + half) % 4
                pA = ps_mm()
                if e in (0, 1):
                    dr6(pA, fw1_8, hc, x1_t)
                    nc.scalar.activation(hout, pA[:], AF.Relu, scale=IWS,
                                         bias=fb1_sb[:, hc:hc + 1])
                else:
                    dr6(pA, fw1_8, hc, x1_t,
                        extra=(fb1row_sb, hc, HFC))
                    nc.vector.tensor_scalar(hout, pA[:], IWS, 0.0,
                                            op0=ALU.mult, op1=ALU.max)
        return ha, hb

    def ffn2_block(x1s, hs):
        x1_a, x1_b = x1s
        ha, hb = hs
        r2_a = unit("C2", "r2_a")
        r2_b = unit("D2", "r2_b")
        for x1_t, hts, r2_t in ((x1_a, ha, r2_a), (x1_b, hb, r2_b)):
            for oc in range(FC):
                pA = ps_mm()
                for j in range(4):
                    for i in range(FC // 2):
                        mm(pA[:], fw2_8[:, oc, ds(j * FC + 2 * i, 2), :],
                           hts[j][:, 2 * i:2 * i + 2, :],
                           start=(j == 0 and i == 0), stop=False,
                           perf_mode=DR)
                bias_mm(pA, fb2row_sb, oc, FC)
                nc.vector.scalar_tensor_tensor(
                    r2_t[:, oc, :], pA[:], IWS, x1_t[:, oc, :],
                    op0=ALU.mult, op1=ALU.add)
        return r2_a, r2_b

    def ln2_cosine(mt, r2s):
        r2_a, r2_b = r2s

        def ln2_stats(r2_t, tok):
            pSt = ps_stat()
            pS2 = ps_stat()
            for c in range(FC):
                sq = chunk_t("sq2t")
                nc.scalar.activation(sq[:], r2_t[:, c, :], AF.Square)
                mm(pSt[0:1, :], sl3_sb[:, c, 0:1], r2_t[:, c, :],
                   start=(c == 0), stop=(c == FC - 1))
                mm(pSt[32:33, :], sl3_sb[:, c, 1:2], r2_t[:, c, :],
                   start=(c == 0), stop=(c == FC - 1))
                mm(pSt[64:65, :], sl3_sb[:, c, 2:3], r2_t[:, c, :],
                   start=(c == 0), stop=(c == FC - 1))
                mm(pSt[96:97, :], sl3_sb[:, c, 0:1], sq[:],
                   start=(c == 0), stop=(c == FC - 1),
                   tile_position=(0, 96))
                mm(pS2[0:1, :], sl3_sb[:, c, 1:2], sq[:],
                   start=(c == 0), stop=(c == FC - 1))
            sz = lane_t("sz" + tok)
            nc.scalar.activation(sz[:], pSt[0:1, :], AF.Copy)
            g2z = lane_t("g2z" + tok)
            nc.scalar.activation(g2z[:], pSt[32:33, :], AF.Copy)
            gbz = lane_t("gbz" + tok)
            nc.scalar.activation(gbz[:], pSt[64:65, :], AF.Copy)
            sq_s = lane_t("sq" + tok)
            nc.scalar.activation(sq_s[:], pSt[96:97, :], AF.Copy)
            g2q = lane_t("g2q" + tok)
            nc.scalar.activation(g2q[:], pS2[0:1, :], AF.Copy)
            return sz, g2z, gbz, sq_s, g2q

        stats_a = ln2_stats(r2_a, "a")
        stats_b = ln2_stats(r2_b, "b")
        pX = ps_head()
        for c in range(FC):
            rr = chunk_t("rr")
            nc.gpsimd.tensor_mul(rr[:], r2_a[:, c, :], r2_b[:, c, :])
            mm(pX[:], sl3_sb[:, c, 1:2], rr[:],
               start=(c == 0), stop=(c == FC - 1))

        def ln2_lane(stats, tok, v):
            sz, g2z, gbz, sq_s, g2q = stats
            muz = lane_t("muz" + tok)
            v.tensor_scalar_mul(muz[:], sz[:], 1.0 / D)
            var = lane_t("var2" + tok)
            v.tensor_mul(var[:], muz[:], muz[:])
            v.scalar_tensor_tensor(var[:], sq_s[:], 1.0 / D,
                                   var[:], op0=ALU.mult, op1=ALU.subtract)
            rstd = lane_t("rstd2" + tok)
            v.tensor_scalar_add(var[:], var[:], EPS_LN)
            nc.scalar.activation(rstd[:], var[:], AF.Sqrt)
            nc.vector.reciprocal(rstd[:], rstd[:])
            return muz, rstd, g2z, gbz, g2q

        def gbt(mu, rstd, gbz, name, v):
            o_t = lane_t(name)
            v.tensor_scalar_mul(o_t[:], mu[:], s_gb)
            v.tensor_tensor(o_t[:], gbz[:], o_t[:], op=ALU.subtract)
            v.tensor_mul(o_t[:], o_t[:], rstd[:])
            return o_t

        def normsq(mu, rstd, g2z, g2q, gbt_t, name, v):
            o_t = lane_t(name)
            v.tensor_scalar_mul(o_t[:], mu[:], s_g2)
            v.scalar_tensor_tensor(o_t[:], g2z[:], -2.0, o_t[:],
                                   op0=ALU.mult, op1=ALU.add)
            v.tensor_mul(o_t[:], o_t[:], mu[:])
            v.tensor_add(o_t[:], o_t[:], g2q[:])
            v.tensor_mul(o_t[:], o_t[:], rstd[:])
            v.tensor_mul(o_t[:], o_t[:], rstd[:])
            v.scalar_tensor_tensor(o_t[:], gbt_t[:], 2.0, o_t[:],
                                   op0=ALU.mult, op1=ALU.add)
            v.tensor_scalar_add(o_t[:], o_t[:], s_bb)
            return o_t

        mua, rsta, g2za, gbza, g2qa = ln2_lane(stats_a, "a", nc.vector)
        mub2, rstb, g2zb, gbzb, g2qb = ln2_lane(stats_b, "b", nc.vector)
        gbta = gbt(mua, rsta, gbza, "gbta", nc.vector)
        gbtb = gbt(mub2, rstb, gbzb, "gbtb", nc.vector)
        n2a = normsq(mua, rsta, g2za, g2qa, gbta, "n2a", nc.vector)
        n2b = normsq(mub2, rstb, g2zb, g2qb, gbtb, "n2b", nc.vector)

        d01 = lane_t("d01")
        nc.vector.tensor_scalar_mul(d01[:], mub2[:], s_g2)
        nc.vector.tensor_tensor(d01[:], d01[:], g2zb[:], op=ALU.subtract)
        nc.vector.tensor_mul(d01[:], d01[:], mua[:])
        t2 = lane_t("t2")
        nc.vector.tensor_mul(t2[:], mub2[:], g2za[:])
        nc.vector.tensor_tensor(d01[:], d01[:], t2[:], op=ALU.subtract)
        nc.vector.tensor_tensor(d01[:], pX[:], d01[:], op=ALU.add)
        nc.vector.tensor_mul(d01[:], d01[:], rsta[:])
        nc.vector.tensor_mul(d01[:], d01[:], rstb[:])
        nc.vector.tensor_add(d01[:], d01[:], gbta[:])
        nc.vector.tensor_add(d01[:], d01[:], gbtb[:])
        nc.vector.tensor_scalar_add(d01[:], d01[:], s_bb)

        den = lane_t("den")
        nc.scalar.activation(n2a[:], n2a[:], AF.Sqrt)
        nc.vector.tensor_scalar_max(n2a[:], n2a[:], EPS_COS)
        nc.scalar.activation(n2b[:], n2b[:], AF.Sqrt)
        nc.vector.tensor_scalar_max(n2b[:], n2b[:], EPS_COS)
        nc.vector.tensor_mul(den[:], n2a[:], n2b[:])
        nc.vector.reciprocal(den[:], den[:])
        atg_sl = lane_t("atg_sl")
        nc.vector.tensor_mul(atg_sl[:], d01[:], den[:])
        sp.dma_start(t["out"][1:2, ts(mt, NP)], atg_sl[:])

    # pipelined driver: A(t+1) emitted inside B(t)
    prep = prepare(0)
    ar = a_stage(0, prep)
    x1s = ln1_block(ar)
    for mt in range(NMACRO):
        hs = ffn1_block(x1s)
        if mt + 1 < NMACRO:
            prep = prepare(mt + 1)
        r2s = ffn2_block(x1s, hs)
        if mt + 1 < NMACRO:
            ar = a_stage(mt + 1, prep)
        stats_emitted = ln2_cosine_pre = None
        if mt + 1 < NMACRO:
            nxt_x1s = ln1_block(ar)
        ln2_cosine(mt, r2s)
        if mt + 1 < NMACRO:
            x1s = nxt_x1s

# ===================== host side =====================

def _prep_weights(inputs):
    """Host-side weight preprocessing: fp8/bf16 conversions, bias rows."""
    f32 = np.float32
    bf16 = ml_dtypes.bfloat16
    fp8 = ml_dtypes.float8_e4m3

    def fm(w):
        # [in, out] -> [P, in//P, out] feature-major
        return np.ascontiguousarray(
            w.reshape(-1, P, w.shape[1]).transpose(1, 0, 2))

    def strip(w, noc):
        # [in, out] -> [P, noc, (in//P)*P] strip-major (oc outer, ic inner)
        i = w.shape[0] // P
        x = w.reshape(i, P, noc, P).transpose(1, 2, 0, 3)
        return np.ascontiguousarray(x.reshape(P, noc, i * P))

    w = {n: np.asarray(inputs[n], f32) for n in
         ["relik_w1", "relik_b1", "relik_w2", "relik_b2",
          "wq", "bq", "wk", "bk", "wv", "bv", "wo", "bo",
          "ln1_g", "ln1_b", "ffn_w1", "ffn_b1", "ffn_w2", "ffn_b2",
          "ln2_g", "ln2_b", "uni_w1", "uni_b1", "uni_w2", "uni_b2"]}

    q8 = lambda x: (WS * x).astype(fp8)
    out = {
        "wq8": q8(strip(w["wq"], FC)), "wk8": q8(strip(w["wk"], FC)),
        "wv8": q8(strip(w["wv"], FC)), "wo8": q8(strip(w["wo"], FC)),
        "u1b8": q8(strip(w["uni_w1"][D:], FC)),
        "fw18": q8(strip(w["ffn_w1"], HFC)),
        "fw28": q8(strip(w["ffn_w2"], FC)),
        "w1bb": strip(w["relik_w1"][D:], FC).astype(bf16),
        "w1ab": strip(w["relik_w1"][:D], FC).astype(bf16),
        "u2rs64": (WS * w["uni_w2"].sum(1)).reshape(FC, P).T.astype(bf16),
        "b2m": np.full((1, 1), w["uni_b2"].mean(), f32),
        "borow64": (WS * w["bo"]).reshape(FC, P).astype(bf16),
        "fb1row64": (WS * w["ffn_b1"]).reshape(HFC, P).astype(bf16),
        "fb2row64": (WS * w["ffn_b2"]).reshape(FC, P).astype(bf16),
        "relik_w2": w["relik_w2"],
        "relik_b2": w["relik_b2"].reshape(1, 1),
        "ident": np.eye(P, dtype=f32),
        "identb": np.eye(P, dtype=f32).astype(bf16),
        "hmat": np.repeat(np.eye(H, dtype=f32), DH, axis=0).astype(bf16),
        "i8neg": (-np.eye(H, dtype=f32)).astype(bf16),
        "idhfc": np.eye(HFC, dtype=f32).astype(bf16),
    }
    for n in ["relik_b1", "bq", "bk", "bv", "ln1_g", "ln1_b",
              "ln2_g", "ln2_b", "uni_b1", "ffn_b1"]:
        out[n] = w[n]
    return out


def kernel(**inputs):
    f32 = np.float32
    bf16 = ml_dtypes.bfloat16
    txt = np.asarray(inputs["text_embeddings"], f32).reshape(S, D)
    cand_full = np.asarray(
        inputs["candidate_embeddings"], f32).reshape(M * K, D).astype(bf16)
    starts = np.asarray(inputs["mention_starts"], np.int64)
    spans = np.asarray(inputs["span_lengths"], np.int64)
    ends = starts + spans

    # host: cumsum + mention/context means (exact f32, like the reference)
    csum = np.concatenate([np.zeros((1, D), f32), np.cumsum(txt, 0,
                                                            dtype=f32)], 0)
    mention = (csum[ends + 1] - csum[starts]) / (
        spans + 1)[:, None].astype(f32)
    c0 = np.maximum(0, starts - CTX)
    c1 = np.minimum(S - 1, ends + CTX)
    ctx = (csum[c1] - csum[c0]) / (c1 - c0)[:, None].astype(f32)

    consts = _prep_weights(inputs)

    in_maps = []
    for core in range(NCORES):
        sl = slice(core * M_LOC, (core + 1) * M_LOC)
        im = {
            "cand": cand_full[core * PAIRS:(core + 1) * PAIRS],
            "mrow": np.ascontiguousarray(mention[sl]),
            "crow": np.ascontiguousarray(ctx[sl]),
        }
        im.update(consts)
        in_maps.append(im)

    if "nc" not in _NC_CACHE:
        _NC_CACHE["nc"] = _build_nc()
    nc = _NC_CACHE["nc"]

    results = bass_utils.run_bass_kernel_spmd(
        nc, in_maps, core_ids=list(range(NCORES))).results

    out = np.zeros((3, M, K), f32)
    for core in range(NCORES):
        sl = slice(core * M_LOC, (core + 1) * M_LOC)
        out[:, sl, :] = results[core]["out"].reshape(3, M_LOC, K)
    return out


if __name__ == "__main__":
    nc = _build_nc()
    print("built ok")


# revision 43
# speedup vs baseline: 2.0688x; 1.1165x over previous
"""Trainium2 Bass kernel for nn_EntityResolutionProcessor.

Strategy: data-parallel over mentions (M=1024 -> 128/core on 8 cores).
v2: fp8e4 DoubleRow matmuls (x64 weight scale) for qkv/wo/FFN/uni paths
(relik stays bf16 for accuracy); weights SBUF-resident, quantized host-side
and uploaded in fp8/bf16; mention/context means precomputed on host
(cumsum+gather); candidates uploaded bf16; PSUM-evict work spread across
ACT/DVE/Pool engines; biases folded into PSUM via bias-row matmuls where
the evict engine lacks them.
"""

from contextlib import ExitStack

import ml_dtypes
import numpy as np

import concourse.bass as bass
import concourse.mybir as mybir
import concourse.tile as tile
from concourse import bacc, bass_isa, bass_utils
from concourse.bass import ds, ts

S, D, M, K, H = 4096, 768, 1024, 32, 8
DH = D // H
CTX = 10
NCORES = 8
P = 128
FC = D // P                     # 6 feature chunks
HFC = 4 * D // P                # 24 ffn hidden chunks
M_LOC = M // NCORES             # 128 mentions per core
PAIRS = M_LOC * K               # 4096 pairs per core
NP = 512                        # pairs per macro tile
G = NP // K                     # 16 mentions per macro tile
NMACRO = PAIRS // NP            # 8
ISQ = 1.0 / float(np.sqrt(np.float32(DH)))
EPS_LN = 1e-5
EPS_COS = 1e-8
WS = 64.0                       # fp8 weight scale
IWS = 1.0 / WS

F32 = mybir.dt.float32
BF16 = mybir.dt.bfloat16
FP8 = mybir.dt.float8e4
AF = mybir.ActivationFunctionType
ALU = mybir.AluOpType
DR = mybir.MatmulPerfMode.DoubleRow

_NC_CACHE = {}


def _gk(ap):
    """view a [128, NP] AP as [128, G, K]"""
    return ap.rearrange("p (g k) -> p g k", g=G)


def _build_nc():
    nc = bacc.Bacc(
        "TRN2", target_bir_lowering=False, debug=False, num_devices=NCORES
    )

    def inp(name, shape, dtype=F32):
        return nc.dram_tensor(name, list(shape), dtype, kind="ExternalInput").ap()

    t = {}
    t["cand"] = inp("cand", [PAIRS, D], BF16)
    t["mrow"] = inp("mrow", [P, D], BF16)  # mention means (row-major)
    t["crow"] = inp("crow", [P, D], BF16)  # context means
    t["sums3"] = inp("sums3", [8, 3])      # [s_g2, s_gb, s_bb] x 8 rows
    t["identb"] = inp("identb", [P, P], BF16)
    t["hmat"] = inp("hmat", [D, H], BF16)  # head indicator
    t["i8neg"] = inp("i8neg", [H, H], BF16)
    t["idhfc"] = inp("idhfc", [HFC, HFC], BF16)
    # bias rows (x64 for psum folds), bf16 [1, X]
    t["borow64"] = inp("borow64", [FC, P], BF16)
    t["fb1row64"] = inp("fb1row64", [HFC, P], BF16)
    t["fb2row64"] = inp("fb2row64", [FC, P], BF16)
    # pre-quantized weights (host side): [P, noc, nic*P]
    t["wq8"] = inp("wq8", [P, FC, D], FP8)
    t["wk8"] = inp("wk8", [P, FC, D], FP8)
    t["wv8"] = inp("wv8", [P, FC, D], FP8)
    t["wo8"] = inp("wo8", [P, FC, D], FP8)
    t["u1b8"] = inp("u1b8", [P, FC, D], FP8)
    t["fw18"] = inp("fw18", [P, HFC, D], FP8)
    t["fw28"] = inp("fw28", [P, FC, 4 * D], FP8)
    t["w1bb"] = inp("w1bb", [P, FC, D], BF16)
    t["w1ab"] = inp("w1ab", [P, FC, D], BF16)
    t["u2rs64"] = inp("u2rs64", [P, FC], BF16)
    t["b2m"] = inp("b2m", [1, 1])

    t["ffn_b1"] = inp("ffn_b1", [4 * D])
    for n, shp in [("relik_b1", [D]), ("relik_w2", [D, 1]),
                   ("relik_b2", [1, 1]),
                   ("bq", [D]), ("bk", [D]), ("bv", [D]),
                   ("ln1_g", [D]), ("ln1_b", [D]),
                   ("ln2_g", [D]), ("ln2_b", [D]),
                   ("uni_b1", [D])]:
        t[n] = inp(n, shp)

    t["out"] = nc.dram_tensor("out", [3, PAIRS], F32, kind="ExternalOutput").ap()

    with tile.TileContext(nc) as tc:
        _body(nc, tc, t)
    nc.compile()
    return nc


def _body(nc, tc, t):
    with ExitStack() as _ctx:
        _body_inner(nc, tc, t, _ctx)


def _vec6(v_ap, n=FC):
    return v_ap.rearrange("(i p) -> p i", p=P)


def _body_inner(nc, tc, t, _ctx):
    mm = lambda *a, **k: nc.tensor.matmul(*a, **k)

    # ---------------- pools ----------------
    psum = _ctx.enter_context(tc.tile_pool(name="psum", bufs=1, space="PSUM"))
    res = _ctx.enter_context(tc.tile_pool(name="res", bufs=1))

    def ps_mm(shape=(P, NP), dtype=F32):
        return psum.tile(list(shape), dtype, tag="mm", bufs=4,
                         padded_shape=[P, NP], name="ps_mm")

    def ps_score():
        return psum.tile([8, NP], F32, tag="score", bufs=1, name="ps_score")

    def ps_stat():
        # stats tile: MM groups land at base partitions 0/32/64/96
        return psum.tile([P, NP], F32, tag="stat", bufs=2, name="ps_stat")

    def ps_head():
        return psum.tile([1, NP], F32, tag="head", bufs=1, name="ps_head")

    # ---------------- resident constants ----------------
    sp = nc.sync

    def load_res(name, ap_src, shape, dtype=F32, q=sp):
        tl = res.tile(list(shape), dtype, name=name)
        q.dma_start(tl[:], ap_src)
        return tl

    identb_sb = load_res("identb_sb", t["identb"][:], [P, P], BF16)
    i8neg_sb = load_res("i8neg_sb", t["i8neg"][:], [H, H], BF16)
    h_sb = load_res("h_sb", t["hmat"].rearrange("(c p) h -> p c h", p=P),
                    [P, FC, H], BF16)
    ht_sb = load_res("ht_sb", t["hmat"].rearrange("(c p) h -> h c p", p=P),
                     [H, FC, P], BF16)
    negh_sb = res.tile([P, FC, H], BF16, name="negh_sb")
    nc.vector.tensor_scalar_mul(negh_sb[:], h_sb[:], -1.0)

    # bias rows for psum folds: row c at partition c (lhsT slice [c:c+1])
    borow_sb = load_res("borow_sb", t["borow64"][:], [FC, P], BF16)
    fb1row_sb = load_res("fb1row_sb", t["fb1row64"][:], [HFC, P], BF16)
    fb2row_sb = load_res("fb2row_sb", t["fb2row64"][:], [FC, P], BF16)

    bq_sb = load_res("bq_sb", _vec6(t["bq"]), [P, FC])
    bk_sb = load_res("bk_sb", _vec6(t["bk"]), [P, FC])
    bv_sb = load_res("bv_sb", _vec6(t["bv"]), [P, FC])
    rb1_sb = load_res("rb1_sb", _vec6(t["relik_b1"]), [P, FC])
    ub1_sb = load_res("ub1_sb", _vec6(t["uni_b1"]), [P, FC])
    fb1_sb = load_res("fb1_sb", _vec6(t["ffn_b1"], HFC), [P, HFC])
    l1g_sb = load_res("l1g_sb", _vec6(t["ln1_g"]), [P, FC])
    l1b_sb = load_res("l1b_sb", _vec6(t["ln1_b"]), [P, FC])
    l2g_sb = load_res("l2g_sb", _vec6(t["ln2_g"]), [P, FC])
    l2b_sb = load_res("l2b_sb", _vec6(t["ln2_b"]), [P, FC])
    rw2_sb = load_res("rw2_sb",
                      t["relik_w2"].rearrange("(c p) o -> p c o", p=P),
                      [P, FC, 1], BF16, q=nc.gpsimd)
    rb2_sb = load_res("rb2_sb", t["relik_b2"][:], [1, 1])
    u2rs_sb = load_res("u2rs_sb", t["u2rs64"][:, :, None], [P, FC, 1], BF16)
    b2m_sb = load_res("b2m_sb", t["b2m"][:], [1, 1])

    # resident weights [P, noc, nic, P]; DR lhsT slice [:, oc, 2i:2i+2, :]
    def load_w(name, nic=FC, noc=FC, dtype=FP8):
        tl = res.tile([P, noc, nic, P], dtype, name=name + "_sb")
        sp.dma_start(tl[:], t[name].rearrange("p o (i q) -> p o i q", q=P))
        return tl

    wq8 = load_w("wq8")
    wk8 = load_w("wk8")
    wv8 = load_w("wv8")
    wo8 = load_w("wo8")
    u1b8 = load_w("u1b8")
    fw2_8 = load_w("fw28", nic=HFC)
    w1b_b16 = load_w("w1bb", dtype=BF16)

    ones_sb = res.tile([P, 1], BF16, name="ones_sb")
    nc.vector.memset(ones_sb[:], 1.0)
    ones_row = res.tile([1, P], BF16, name="ones_row")
    nc.vector.memset(ones_row[:], 1.0)
    idh_sb = load_res("idh_sb", t["idhfc"][:], [HFC, HFC], BF16)

    def bias_mm(pA, row_sb, r, nrow, n=NP):
        mm(pA[:], row_sb[:],
           idh_sb[:nrow, r, None].to_broadcast([nrow, n]),
           start=False, stop=True)

    # stats lhsT [128, 6, 3]: cols = [1, g2^2, g2*b2] per feature chunk
    sl3_sb = res.tile([P, FC, 3], BF16, name="sl3_sb")
    g2sq_sb = res.tile([P, FC], F32, name="g2sq_sb")
    g2b2_sb = res.tile([P, FC], F32, name="g2b2_sb")
    nc.vector.tensor_mul(g2sq_sb[:], l2g_sb[:], l2g_sb[:])
    nc.vector.tensor_mul(g2b2_sb[:], l2g_sb[:], l2b_sb[:])
    for c in range(FC):
        nc.vector.tensor_copy(sl3_sb[:, c, 0:1], ones_sb[:])
        nc.vector.tensor_copy(sl3_sb[:, c, 1:2], g2sq_sb[:, c:c + 1])
        nc.vector.tensor_copy(sl3_sb[:, c, 2:3], g2b2_sb[:, c:c + 1])
        pass

    sums3_sb = load_res("sums3_sb", t["sums3"][:], [8, 3])
    s_g2 = sums3_sb[0:8, 0:1]
    s_gb = sums3_sb[0:8, 1:2]
    s_bb = sums3_sb[0:8, 2:3]
    # batched cosine stats, one partition per macro tile
    st_names = ["sza", "g2za", "gbza", "sqa", "g2qa",
                "szb", "g2zb", "gbzb", "sqb", "g2qb", "pXs"]
    st_all = {n: res.tile([8, NP], BF16, name="st_" + n) for n in st_names}

    # per-mention outputs (feature-major [128, 6, 128])
    womv = res.tile([P, FC, P], BF16, name="womv")   # wo@m_v + m_T + bo
    m_Tb = res.tile([P, FC, P], BF16, name="m_Tb")   # bf16: relik rhs
    m_T8 = res.tile([P, FC, P], FP8, name="m_T8")    # fp8: DR rhs
    c_T8 = res.tile([P, FC, P], FP8, name="c_T8")
    m_q = res.tile([P, FC, P], BF16, name="m_q")
    m_k = res.tile([P, FC, P], BF16, name="m_k")
    m_v = res.tile([P, FC, P], BF16, name="m_v")
    m_relik = res.tile([P, FC, P], BF16, name="m_relik")
    c_uni = res.tile([P, FC, P], BF16, name="c_uni")
    s_aa_sb = res.tile([H, P], BF16, name="s_aa_sb")

    # ================= phase 0: transposes + per-mention =================
    with tc.tile_pool(name="p0", bufs=1) as p0:
        mrow = p0.tile([P, D], BF16, name="mrow_t")
        crow = p0.tile([P, D], BF16, name="crow_t")
        sp.dma_start(mrow[:], t["mrow"][:])
        sp.dma_start(crow[:], t["crow"][:])
        m_T = p0.tile([P, FC, P], BF16, name="m_T")
        for fc in range(FC):
            pT = ps_mm((P, P), BF16)
            nc.tensor.transpose(pT[:], mrow[:, ts(fc, P)], identb_sb[:])
            nc.vector.tensor_copy(m_T[:, fc, :], pT[:])
            nc.any.tensor_copy(m_Tb[:, fc, :], pT[:])
            nc.scalar.activation(m_T8[:, fc, :], pT[:], AF.Copy)
            pT2 = ps_mm((P, P), BF16)
            nc.tensor.transpose(pT2[:], crow[:, ts(fc, P)], identb_sb[:])
            nc.scalar.activation(c_T8[:, fc, :], pT2[:], AF.Copy)

        # per-mention projections: q/k/v/uni via fp8 DR; relik via bf16
        for w8, b_sb, out_t, src8 in (
            (wq8, bq_sb, m_q, m_T8),
            (wk8, bk_sb, m_k, m_T8),
            (wv8, bv_sb, m_v, m_T8),
            (u1b8, ub1_sb, c_uni, c_T8),
        ):
            for oc in range(FC):
                pA = ps_mm((P, P))
                for i in range(FC // 2):
                    mm(pA[:], w8[:, oc, 2 * i:2 * i + 2, :],
                       src8[:, 2 * i:2 * i + 2, :],
                       start=(i == 0), stop=(i == FC // 2 - 1), perf_mode=DR)
                nc.scalar.activation(out_t[:, oc, :], pA[:], AF.Identity,
                                     bias=b_sb[:, oc:oc + 1], scale=IWS)
        m_v8 = p0.tile([P, FC, P], FP8, name="m_v8")
        for c in range(FC):
            nc.scalar.activation(m_v8[:, c, :], m_v[:, c, :], AF.Copy)
        for oc in range(FC):
            pA = ps_mm((P, P))
            for i in range(FC // 2):
                mm(pA[:], wo8[:, oc, 2 * i:2 * i + 2, :],
                   m_v8[:, 2 * i:2 * i + 2, :],
                   start=(i == 0), stop=False, perf_mode=DR)
            bias_mm(pA, borow_sb, oc, FC, n=P)
            nc.vector.scalar_tensor_tensor(
                womv[:, oc, :], pA[:], IWS, m_T[:, oc, :],
                op0=ALU.mult, op1=ALU.add)
        w1a_st = p0.tile([P, FC, FC, P], BF16, name="w1a_st")
        sp.dma_start(w1a_st[:],
                     t["w1ab"].rearrange("p o (i q) -> p o i q", q=P))
        for oc in range(FC):
            pA = ps_mm((P, P))
            for ic in range(FC):
                mm(pA[:], w1a_st[:, oc, ic, :], m_Tb[:, ic, :],
                   start=(ic == 0), stop=(ic == FC - 1))
            nc.scalar.activation(m_relik[:, oc, :], pA[:], AF.Identity,
                                 bias=rb1_sb[:, oc:oc + 1])

        # s_aa [8, 128]
        mprod = p0.tile([P, FC, P], BF16, name="mprod")
        for c in range(FC):
            nc.vector.tensor_mul(mprod[:, c, :], m_q[:, c, :], m_k[:, c, :])
        pS = ps_score()
        for c in range(FC):
            mm(pS[:, :P], h_sb[:, c, :], mprod[:, c, :],
               start=(c == 0), stop=(c == FC - 1))
        nc.any.tensor_copy(s_aa_sb[:], pS[:, :P])

    # ================= pools for the main phase =================
    act = _ctx.enter_context(tc.tile_pool(name="act", bufs=1))
    lane = _ctx.enter_context(tc.tile_pool(name="lane", bufs=1))

    def unit(tag, name, dtype=BF16, bufs=1):
        return act.tile([P, FC, NP], dtype, tag=tag, bufs=bufs, name=name)

    def chunk_t(name, dtype=BF16):
        return act.tile([P, NP], dtype, tag="tt", bufs=4, name=name)

    def dr6(pA, w8, oc, rhs8, extra=None):
        """3 DoubleRow matmuls contracting 6 feature chunks (+ optional
        bias-row matmul appended to the accumulation group)."""
        n = FC // 2
        for i in range(n):
            mm(pA[:], w8[:, oc, 2 * i:2 * i + 2, :],
               rhs8[:, 2 * i:2 * i + 2, :],
               start=(i == 0), stop=(i == n - 1 and extra is None),
               perf_mode=DR)
        if extra is not None:
            row_sb, r, nrow = extra
            bias_mm(pA, row_sb, r, nrow)

    # ============ macro-tile loop (2-deep software pipeline) ============
    def lane_t(name):
        return lane.tile([1, NP], F32, tag="lt", bufs=7, name=name)

    def mview_of(mt):
        gsl = ds(mt * G, G)

        def mview(mt_tile, c):
            return mt_tile[:, c, gsl, None].to_broadcast([P, G, K])
        return gsl, mview

    def prepare(mt):
        """candidate DMA + transpose + k/v projections for tile mt."""
        cand_rm = act.tile([P, 4, D], BF16, tag="cand_rm", bufs=1,
                           name="cand_rm")
        sp.dma_start(
            cand_rm[:],
            t["cand"].rearrange("(q p) d -> p q d", p=P)[:, ds(4 * mt, 4), :])
        candT = unit("candT", "candT")
        candT8 = unit("candT8", "candT8", FP8)
        for fc in range(FC):
            pT = ps_mm(dtype=BF16)
            for pc in range(4):
                nc.tensor.transpose(pT[:, ts(pc, P)],
                                    cand_rm[:, pc, ts(fc, P)], identb_sb[:])
            nc.scalar.activation(candT[:, fc, :], pT[:], AF.Copy)
            nc.gpsimd.tensor_copy(candT8[:, fc, :], candT[:, fc, :])
        k_b = unit("B", "k_b")
        v_b = unit("C", "v_b")
        for w8, b_sb, out_t in ((wk8, bk_sb, k_b), (wv8, bv_sb, v_b)):
            for oc in range(FC):
                pA = ps_mm()
                dr6(pA, w8, oc, candT8)
                nc.scalar.activation(out_t[:, oc, :], pA[:], AF.Identity,
                                     bias=b_sb[:, oc:oc + 1], scale=IWS)
        return candT, candT8, k_b, v_b

    def a_stage(mt, prep):
        """relik/uni heads, attention scores+outputs, wo residual."""
        candT, candT8, k_b, v_b = prep
        gsl, mview = mview_of(mt)

        # relik head (bf16)
        h_r = unit("hh", "h_r", bufs=2)
        for oc in range(FC):
            pA = ps_mm()
            for ic in range(FC):
                mm(pA[:], w1b_b16[:, oc, ic, :], candT[:, ic, :],
                   start=(ic == 0), stop=(ic == FC - 1))
            nc.vector.tensor_tensor(_gk(h_r[:, oc, :]), _gk(pA[:]),
                                    mview(m_relik, oc), op=ALU.add)
            nc.scalar.activation(h_r[:, oc, :], h_r[:, oc, :], AF.Relu)
        pH = ps_head()
        for c in range(FC):
            mm(pH[:], rw2_sb[:, c, :], h_r[:, c, :],
               start=(c == 0), stop=(c == FC - 1))
        osl_r = lane_t("osl_r")
        nc.scalar.activation(osl_r[:], pH[:], AF.Identity, bias=rb2_sb[:])
        sp.dma_start(t["out"][0:1, ts(mt, NP)], osl_r[:])

        # uni head (fp8 DR)
        h_u = unit("hu8", "h_u", FP8)
        for oc in range(FC):
            pA = ps_mm()
            dr6(pA, u1b8, oc, candT8)
            hp = chunk_t("hp")
            nc.vector.scalar_tensor_tensor(
                _gk(hp[:]), _gk(pA[:]), IWS, mview(c_uni, oc),
                op0=ALU.mult, op1=ALU.add)
            nc.scalar.activation(h_u[:, oc, :], hp[:], AF.Relu)
        pH2 = ps_head()
        for c in range(FC):
            mm(pH2[:], u2rs_sb[:, c, :], h_u[:, c, :],
               start=(c == 0), stop=(c == FC - 1))
        osl_u = lane_t("osl_u")
        nc.scalar.activation(osl_u[:], pH2[:], AF.Sigmoid, bias=b2m_sb[:],
                             scale=1.0 / (WS * D))
        sp.dma_start(t["out"][2:3, ts(mt, NP)], osl_u[:])

        # attention scores A->B
        pAB = ps_score()
        for c in range(FC):
            pr1 = chunk_t("pr1")
            nc.gpsimd.tensor_tensor(_gk(pr1[:]), _gk(k_b[:, c, :]),
                                    mview(m_q, c), op=ALU.mult)
            mm(pAB[:], h_sb[:, c, :], pr1[:], start=(c == 0), stop=False)
        mm(pAB[:], i8neg_sb[:],
           s_aa_sb[:, gsl, None].to_broadcast([H, G, K]),
           start=False, stop=True)
        p_ab = act.tile([H, NP], BF16, tag="p_ab", bufs=1, name="p_ab")
        nc.scalar.activation(p_ab[:], pAB[:], AF.Sigmoid, scale=ISQ)

        # attention scores B->A
        pBA = psum.tile([8, NP], F32, tag="stat", bufs=2,
                        padded_shape=[P, NP], name="pBA")
        first = True
        for c in range(FC):
            pQ = ps_mm()
            dr6(pQ, wq8, c, candT8)
            q_c = chunk_t("q_c")
            nc.scalar.activation(q_c[:], pQ[:], AF.Identity,
                                 bias=bq_sb[:, c:c + 1], scale=IWS)
            pr2 = chunk_t("pr2")
            nc.gpsimd.tensor_tensor(_gk(pr2[:]), _gk(q_c[:]), mview(m_k, c),
                                    op=ALU.mult)
            mm(pBA[:], h_sb[:, c, :], pr2[:], start=first, stop=False)
            first = False
            pr3 = chunk_t("pr3")
            nc.vector.tensor_mul(pr3[:], q_c[:], k_b[:, c, :])
            mm(pBA[:], negh_sb[:, c, :], pr3[:],
               start=False, stop=(c == FC - 1))
        p_ba = act.tile([H, NP], BF16, tag="p_ba", bufs=1, name="p_ba")
        nc.scalar.activation(p_ba[:], pBA[:], AF.Sigmoid, scale=ISQ)

        # attention outputs (fp8 for wo rhs)
        o_a = unit("F8", "o_a", FP8)
        o_b = unit("G8", "o_b", FP8)
        for c in range(FC):
            dv = chunk_t("dv")
            nc.gpsimd.tensor_tensor(_gk(dv[:]), _gk(v_b[:, c, :]),
                                    mview(m_v, c), op=ALU.subtract)
            pBC = ps_mm()
            mm(pBC[:], ht_sb[:, c, :], p_ab[:], start=True, stop=True)
            nc.vector.tensor_mul(o_a[:, c, :], pBC[:], dv[:])
            pBC2 = ps_mm()
            mm(pBC2[:], ht_sb[:, c, :], p_ba[:], start=True, stop=True)
            pdv2 = chunk_t("pdv2")
            nc.vector.tensor_mul(pdv2[:], pBC2[:], dv[:])
            nc.gpsimd.tensor_tensor(o_b[:, c, :], v_b[:, c, :], pdv2[:],
                                    op=ALU.subtract)

        # wo + residual
        r_a = unit("hh", "r_a", bufs=2)
        r_b = unit("rb", "r_b")
        for oc in range(FC):
            pA = ps_mm()
            dr6(pA, wo8, oc, o_a)
            nc.vector.scalar_tensor_tensor(
                _gk(r_a[:, oc, :]), _gk(pA[:]), IWS, mview(womv, oc),
                op0=ALU.mult, op1=ALU.add)
            pB = ps_mm()
            dr6(pB, wo8, oc, o_b, extra=(borow_sb, oc, FC))
            nc.vector.scalar_tensor_tensor(
                r_b[:, oc, :], pB[:], IWS, candT[:, oc, :],
                op0=ALU.mult, op1=ALU.add)
        return candT, r_a, r_b

    def ln1_block(ar):
        candT, r_a, r_b = ar
        x1_a = unit("A8", "x1_a", FP8)
        x1_b = unit("B8", "x1_b", FP8)
        ln1 = []
        for r_t, tok in ((r_a, "a"), (r_b, "b")):
            pSt = ps_stat()
            for c in range(FC):
                sq = chunk_t("sq")
                nc.gpsimd.tensor_mul(sq[:], r_t[:, c, :], r_t[:, c, :])
                mm(pSt[0:1, :], ones_sb[:], r_t[:, c, :],
                   start=(c == 0), stop=(c == FC - 1))
                mm(pSt[32:33, :], ones_sb[:], sq[:],
                   start=(c == 0), stop=(c == FC - 1))
            ln1.append(pSt)
        mr = []
        for pSt, tok in zip(ln1, "ab"):
            mu = lane_t("mu" + tok)
            nc.vector.tensor_scalar_mul(mu[:], pSt[0:1, :], 1.0 / D)
            var = lane_t("var" + tok)
            nc.vector.tensor_mul(var[:], mu[:], mu[:])
            nc.vector.scalar_tensor_tensor(var[:], pSt[32:33, :], 1.0 / D,
                                           var[:], op0=ALU.mult,
                                           op1=ALU.subtract)
            rstd = lane_t("rstd" + tok)
            nc.vector.tensor_scalar_add(var[:], var[:], EPS_LN)
            nc.scalar.activation(rstd[:], var[:], AF.Sqrt)
            nc.vector.reciprocal(rstd[:], rstd[:])
            mubf = act.tile([1, NP], BF16, tag="mubf", bufs=1, name="mubf")
            rstdbf = act.tile([1, NP], BF16, tag="rstdbf", bufs=1,
                              name="rstdbf")
            nc.scalar.activation(mubf[:], mu[:], AF.Copy)
            nc.scalar.activation(rstdbf[:], rstd[:], AF.Copy)
            mu_sb = act.tile([P, NP], BF16, tag="mu_sb" + tok, bufs=1,
                             name="mu_sb")
            rstd_sb = act.tile([P, NP], BF16, tag="rstd_sb" + tok, bufs=1,
                               name="rstd_sb")
            rstd_bc = ps_mm()
            mm(rstd_bc[:], ones_row[:], rstdbf[:], start=True, stop=True)
            nc.scalar.activation(rstd_sb[:], rstd_bc[:], AF.Copy)
            mu_bc = ps_mm()
            mm(mu_bc[:], ones_row[:], mubf[:], start=True, stop=True)
            nc.scalar.activation(mu_sb[:], mu_bc[:], AF.Copy)
            mr.append((mu_sb, rstd_sb))
        for (r_t, x1_t), (mu_sb, rstd_sb) in zip(
                ((r_a, x1_a), (r_b, x1_b)), mr):
            for c in range(FC):
                t1 = chunk_t("t1")
                nc.gpsimd.tensor_tensor(t1[:], r_t[:, c, :], mu_sb[:],
                                        op=ALU.subtract)
                t2 = chunk_t("t2")
                nc.vector.tensor_mul(t2[:], t1[:], rstd_sb[:])
                nc.vector.tensor_scalar(x1_t[:, c, :], t2[:],
                                        l1g_sb[:, c:c + 1],
                                        l1b_sb[:, c:c + 1],
                                        op0=ALU.mult, op1=ALU.add)
        return x1_a, x1_b

    def ffn1_block(x1s):
        x1_a, x1_b = x1s
        ha = act.tile([P, HFC, NP], FP8, tag="ha8", bufs=1, name="h_a")
        ha = [ha[:, 6 * j:6 * (j + 1), :] for j in range(4)]
        hb = [unit("F8", "hb0", FP8), unit("G8", "hb1", FP8),
              unit("hu8", "hb2", FP8), unit("hh", "hb3", FP8, bufs=2)]
        for q in range(4):
            fq = act.tile([P, FC, FC, P], FP8, tag="fw1q", bufs=2,
                          name="fw1q")
            sp.dma_start(fq[:], t["fw18"].rearrange(
                "p o (i q) -> p o i q", q=P)[:, ds(q * FC, FC), :, :])
            for j in range(FC):
                hc = q * FC + j
                for x1_t, hts, half in ((x1_a, ha, 0), (x1_b, hb, 1)):
                    hout = hts[hc // FC][:, hc % FC, :]
                    e = (hc * 2 + half) % 3
                    pA = ps_mm()
                    if e in (0, 1):
                        dr6(pA, fq, j, x1_t)
                        nc.scalar.activation(hout, pA[:], AF.Relu,
                                             scale=IWS,
                                             bias=fb1_sb[:, hc:hc + 1])
                    else:
                        dr6(pA, fq, j, x1_t,
                            extra=(fb1row_sb, hc, HFC))
                        nc.vector.tensor_scalar(hout, pA[:], IWS, 0.0,
                                                op0=ALU.mult, op1=ALU.max)
        return ha, hb

    def ffn2_block(x1s, hs):
        x1_a, x1_b = x1s
        ha, hb = hs
        r2_a = unit("C2", "r2_a")
        r2_b = unit("D2", "r2_b")
        for x1_t, hts, r2_t in ((x1_a, ha, r2_a), (x1_b, hb, r2_b)):
            for oc in range(FC):
                pA = ps_mm()
                for j in range(4):
                    for i in range(FC // 2):
                        mm(pA[:], fw2_8[:, oc, ds(j * FC + 2 * i, 2), :],
                           hts[j][:, 2 * i:2 * i + 2, :],
                           start=(j == 0 and i == 0), stop=False,
                           perf_mode=DR)
                bias_mm(pA, fb2row_sb, oc, FC)
                nc.vector.scalar_tensor_tensor(
                    r2_t[:, oc, :], pA[:], IWS, x1_t[:, oc, :],
                    op0=ALU.mult, op1=ALU.add)
        return r2_a, r2_b

    def ln2_cosine(mt, r2s):
        """per-tile LN2 stats -> bf16 rows of st_all[*] (partition mt)."""
        r2_a, r2_b = r2s

        def ln2_stats(r2_t, tok):
            pSt = ps_stat()
            pS2 = ps_stat()
            for c in range(FC):
                sq = chunk_t("sq2t")
                nc.scalar.activation(sq[:], r2_t[:, c, :], AF.Square)
                mm(pSt[0:1, :], sl3_sb[:, c, 0:1], r2_t[:, c, :],
                   start=(c == 0), stop=(c == FC - 1))
                mm(pSt[32:33, :], sl3_sb[:, c, 1:2], r2_t[:, c, :],
                   start=(c == 0), stop=(c == FC - 1))
                mm(pSt[64:65, :], sl3_sb[:, c, 2:3], r2_t[:, c, :],
                   start=(c == 0), stop=(c == FC - 1))
                mm(pSt[96:97, :], sl3_sb[:, c, 0:1], sq[:],
                   start=(c == 0), stop=(c == FC - 1),
                   tile_position=(0, 96))
                mm(pS2[0:1, :], sl3_sb[:, c, 1:2], sq[:],
                   start=(c == 0), stop=(c == FC - 1))
            for nm, row in (("sz" + tok, pSt[0:1, :]),
                            ("g2z" + tok, pSt[32:33, :]),
                            ("gbz" + tok, pSt[64:65, :]),
                            ("sq" + tok, pSt[96:97, :]),
                            ("g2q" + tok, pS2[0:1, :])):
                lt = lane.tile([1, NP], BF16, tag="lt", bufs=7, name=nm)
                nc.scalar.activation(lt[:], row, AF.Copy)
                sp.dma_start(st_all[nm[:-1] + tok][mt:mt + 1, :], lt[:])

        ln2_stats(r2_a, "a")
        ln2_stats(r2_b, "b")
        pX = ps_head()
        for c in range(FC):
            rr = chunk_t("rr")
            nc.gpsimd.tensor_mul(rr[:], r2_a[:, c, :], r2_b[:, c, :])
            mm(pX[:], sl3_sb[:, c, 1:2], rr[:],
               start=(c == 0), stop=(c == FC - 1))
        ltx = lane.tile([1, NP], BF16, tag="lt", bufs=7, name="pXl")
        nc.scalar.activation(ltx[:], pX[:], AF.Copy)
        sp.dma_start(st_all["pXs"][mt:mt + 1, :], ltx[:])

    def batch_cosine():
        """cosine for all 8 tiles at once on [8, NP] tiles."""
        _lp = _ctx.enter_context(
            nc.allow_low_precision(reason="cosine is scale-invariant"))

        def bt(name):
            return lane.tile([8, NP], BF16, tag="bt", bufs=12, name=name)

        def ln2_lane(tok):
            sz, sq_s = st_all["sz" + tok], st_all["sq" + tok]
            muz = bt("muz" + tok)
            nc.vector.tensor_scalar_mul(muz[:], sz[:], 1.0 / D)
            var = bt("var2" + tok)
            nc.vector.tensor_mul(var[:], muz[:], muz[:])
            nc.vector.scalar_tensor_tensor(var[:], sq_s[:], 1.0 / D,
                                           var[:], op0=ALU.mult,
                                           op1=ALU.subtract)
            rstd = bt("rstd2" + tok)
            nc.vector.tensor_scalar_add(var[:], var[:], EPS_LN)
            nc.scalar.activation(rstd[:], var[:], AF.Sqrt)
            nc.vector.reciprocal(rstd[:], rstd[:])
            return muz, rstd

        def gbt_f(mu, rstd, gbz, name):
            o_t = bt(name)
            nc.vector.tensor_scalar_mul(o_t[:], mu[:], s_gb)
            nc.vector.tensor_tensor(o_t[:], gbz[:], o_t[:], op=ALU.subtract)
            nc.vector.tensor_mul(o_t[:], o_t[:], rstd[:])
            return o_t

        def normsq(mu, rstd, g2z, g2q, gbt_t, name):
            o_t = bt(name)
            nc.vector.tensor_scalar_mul(o_t[:], mu[:], s_g2)
            nc.vector.scalar_tensor_tensor(o_t[:], g2z[:], -2.0, o_t[:],
                                           op0=ALU.mult, op1=ALU.add)
            nc.vector.tensor_mul(o_t[:], o_t[:], mu[:])
            nc.vector.tensor_add(o_t[:], o_t[:], g2q[:])
            nc.vector.tensor_mul(o_t[:], o_t[:], rstd[:])
            nc.vector.tensor_mul(o_t[:], o_t[:], rstd[:])
            nc.vector.scalar_tensor_tensor(o_t[:], gbt_t[:], 2.0, o_t[:],
                                           op0=ALU.mult, op1=ALU.add)
            nc.vector.tensor_scalar_add(o_t[:], o_t[:], s_bb)
            return o_t

        mua, rsta = ln2_lane("a")
        mub2, rstb = ln2_lane("b")
        gbta = gbt_f(mua, rsta, st_all["gbza"], "gbta")
        gbtb = gbt_f(mub2, rstb, st_all["gbzb"], "gbtb")
        n2a = normsq(mua, rsta, st_all["g2za"], st_all["g2qa"], gbta, "n2a")
        n2b = normsq(mub2, rstb, st_all["g2zb"], st_all["g2qb"], gbtb,
                     "n2b")

        d01 = bt("d01")
        nc.vector.tensor_scalar_mul(d01[:], mub2[:], s_g2)
        nc.vector.tensor_tensor(d01[:], d01[:], st_all["g2zb"][:],
                                op=ALU.subtract)
        nc.vector.tensor_mul(d01[:], d01[:], mua[:])
        t2 = bt("t2")
        nc.vector.tensor_mul(t2[:], mub2[:], st_all["g2za"][:])
        nc.vector.tensor_tensor(d01[:], d01[:], t2[:], op=ALU.subtract)
        nc.vector.tensor_tensor(d01[:], st_all["pXs"][:], d01[:],
                                op=ALU.add)
        nc.vector.tensor_mul(d01[:], d01[:], rsta[:])
        nc.vector.tensor_mul(d01[:], d01[:], rstb[:])
        nc.vector.tensor_add(d01[:], d01[:], gbta[:])
        nc.vector.tensor_add(d01[:], d01[:], gbtb[:])
        nc.vector.tensor_scalar_add(d01[:], d01[:], s_bb)

        den = bt("den")
        nc.scalar.activation(n2a[:], n2a[:], AF.Sqrt)
        nc.vector.tensor_scalar_max(n2a[:], n2a[:], EPS_COS)
        nc.scalar.activation(n2b[:], n2b[:], AF.Sqrt)
        nc.vector.tensor_scalar_max(n2b[:], n2b[:], EPS_COS)
        nc.vector.tensor_mul(den[:], n2a[:], n2b[:])
        nc.vector.reciprocal(den[:], den[:])
        atg_sl = bt("atg_sl")
        nc.vector.tensor_mul(atg_sl[:], d01[:], den[:])
        nc.gpsimd.dma_start(
            t["out"][1:2, :].rearrange("o (m n) -> (o m) n", n=NP),
            atg_sl[:])

    # pipelined driver: A(t+1) emitted inside B(t)
    prep = prepare(0)
    ar = a_stage(0, prep)
    x1s = ln1_block(ar)
    for mt in range(NMACRO):
        hs = ffn1_block(x1s)
        if mt + 1 < NMACRO:
            prep = prepare(mt + 1)
        r2s = ffn2_block(x1s, hs)
        if mt + 1 < NMACRO:
            ar = a_stage(mt + 1, prep)
        stats_emitted = ln2_cosine_pre = None
        if mt + 1 < NMACRO:
            nxt_x1s = ln1_block(ar)
        ln2_cosine(mt, r2s)
        if mt + 1 < NMACRO:
            x1s = nxt_x1s
    batch_cosine()

# ===================== host side =====================

def _prep_weights(inputs):
    """Host-side weight preprocessing: fp8/bf16 conversions, bias rows."""
    f32 = np.float32
    bf16 = ml_dtypes.bfloat16
    fp8 = ml_dtypes.float8_e4m3

    def fm(w):
        # [in, out] -> [P, in//P, out] feature-major
        return np.ascontiguousarray(
            w.reshape(-1, P, w.shape[1]).transpose(1, 0, 2))

    def strip(w, noc):
        # [in, out] -> [P, noc, (in//P)*P] strip-major (oc outer, ic inner)
        i = w.shape[0] // P
        x = w.reshape(i, P, noc, P).transpose(1, 2, 0, 3)
        return np.ascontiguousarray(x.reshape(P, noc, i * P))

    w = {n: np.asarray(inputs[n], f32) for n in
         ["relik_w1", "relik_b1", "relik_w2", "relik_b2",
          "wq", "bq", "wk", "bk", "wv", "bv", "wo", "bo",
          "ln1_g", "ln1_b", "ffn_w1", "ffn_b1", "ffn_w2", "ffn_b2",
          "ln2_g", "ln2_b", "uni_w1", "uni_b1", "uni_w2", "uni_b2"]}

    q8 = lambda x: (WS * x).astype(fp8)
    out = {
        "wq8": q8(strip(w["wq"], FC)), "wk8": q8(strip(w["wk"], FC)),
        "wv8": q8(strip(w["wv"], FC)), "wo8": q8(strip(w["wo"], FC)),
        "u1b8": q8(strip(w["uni_w1"][D:], FC)),
        "fw18": q8(strip(w["ffn_w1"], HFC)),
        "fw28": q8(strip(w["ffn_w2"], FC)),
        "w1bb": strip(w["relik_w1"][D:], FC).astype(bf16),
        "w1ab": strip(w["relik_w1"][:D], FC).astype(bf16),
        "u2rs64": (WS * w["uni_w2"].sum(1)).reshape(FC, P).T.astype(bf16),
        "b2m": np.full((1, 1), w["uni_b2"].mean(), f32),
        "sums3": np.tile(np.array([[np.sum(w["ln2_g"] ** 2),
                                    np.sum(w["ln2_g"] * w["ln2_b"]),
                                    np.sum(w["ln2_b"] ** 2)]], f32), (8, 1)),
        "borow64": (WS * w["bo"]).reshape(FC, P).astype(bf16),
        "fb1row64": (WS * w["ffn_b1"]).reshape(HFC, P).astype(bf16),
        "fb2row64": (WS * w["ffn_b2"]).reshape(FC, P).astype(bf16),
        "relik_w2": w["relik_w2"],
        "relik_b2": w["relik_b2"].reshape(1, 1),
        "identb": np.eye(P, dtype=f32).astype(bf16),
        "hmat": np.repeat(np.eye(H, dtype=f32), DH, axis=0).astype(bf16),
        "i8neg": (-np.eye(H, dtype=f32)).astype(bf16),
        "idhfc": np.eye(HFC, dtype=f32).astype(bf16),
    }
    for n in ["relik_b1", "bq", "bk", "bv", "ln1_g", "ln1_b",
              "ln2_g", "ln2_b", "uni_b1", "ffn_b1"]:
        out[n] = w[n]
    return out


def kernel(**inputs):
    f32 = np.float32
    bf16 = ml_dtypes.bfloat16
    txt = np.asarray(inputs["text_embeddings"], f32).reshape(S, D)
    cand_full = np.asarray(
        inputs["candidate_embeddings"], f32).reshape(M * K, D).astype(bf16)
    starts = np.asarray(inputs["mention_starts"], np.int64)
    spans = np.asarray(inputs["span_lengths"], np.int64)
    ends = starts + spans

    # host: cumsum + mention/context means (exact f32, like the reference)
    csum = np.concatenate([np.zeros((1, D), f32), np.cumsum(txt, 0,
                                                            dtype=f32)], 0)
    mention = (csum[ends + 1] - csum[starts]) / (
        spans + 1)[:, None].astype(f32)
    c0 = np.maximum(0, starts - CTX)
    c1 = np.minimum(S - 1, ends + CTX)
    ctx = (csum[c1] - csum[c0]) / (c1 - c0)[:, None].astype(f32)

    consts = _prep_weights(inputs)

    in_maps = []
    for core in range(NCORES):
        sl = slice(core * M_LOC, (core + 1) * M_LOC)
        im = {
            "cand": cand_full[core * PAIRS:(core + 1) * PAIRS],
            "mrow": np.ascontiguousarray(mention[sl]).astype(bf16),
            "crow": np.ascontiguousarray(ctx[sl]).astype(bf16),
        }
        im.update(consts)
        in_maps.append(im)

    if "nc" not in _NC_CACHE:
        _NC_CACHE["nc"] = _build_nc()
    nc = _NC_CACHE["nc"]

    results = bass_utils.run_bass_kernel_spmd(
        nc, in_maps, core_ids=list(range(NCORES))).results

    out = np.zeros((3, M, K), f32)
    for core in range(NCORES):
        sl = slice(core * M_LOC, (core + 1) * M_LOC)
        out[:, sl, :] = results[core]["out"].reshape(3, M_LOC, K)
    return out


if __name__ == "__main__":
    nc = _build_nc()
    print("built ok")
